# revision 22
# baseline (speedup 1.0000x reference)
"""MinimalGPT forward on 8 Trainium2 NeuronCores — v2.

Sharding: sequence-parallel transformer + vocab-parallel head (zigzag).
  core c: batch b=c//4, rank r=c%4, owns seq chunks (r, 7-r) = 2x128 tokens.

v2 changes vs v1:
  - bf16 weights + activations end-to-end (fp32 PSUM accumulate, fp32 LN
    stats); halves HBM + collective bytes, enables 128-wide matmuls at
    full PE rate.
  - causal structure: qc1 (chunk r) attends only to chunks 0-3, qc2 only
    to 0..7-r; additive masks applied as identity-matmuls into the score
    PSUM (uniform SPMD program, per-core mask data).
  - softmax denominators ride the AV matmul (ones column packed into V);
    reciprocal via DVE reciprocal_approx_fast on head-pairs.
  - per-layer kv AllGather split into two (one per token chunk), issued
    as soon as that chunk's kv is projected; attention over chunks 0-3
    starts after AG0, hiding most collective latency behind compute.
  - ACT engine uses a single fn table (exp/ln/square/identity); LN rstd
    computed as exp(-0.5*ln(var+eps)).
"""

import math
import os
import numpy as np
import ml_dtypes
from contextlib import ExitStack

import concourse.bass as bass
import concourse.tile as tile
from concourse import bacc, mybir
from concourse.bass_utils import run_bass_kernel_spmd
from concourse.masks import make_identity

f32 = mybir.dt.float32
bf16 = mybir.dt.bfloat16
i32 = mybir.dt.int32
AF = mybir.ActivationFunctionType
OP = mybir.AluOpType

V, D, H, L, F = 32000, 768, 12, 6, 3072
B, S = 2, 1024
P = 128
DK = 64
DCH = D // P           # 6
FCH = F // P           # 24
TOK = 256              # tokens per core (2 chunks of 128)
NCORE, GRP = 8, 4
VPAD = 4096
VCH = VPAD // P        # 32
VSH = V // NCORE       # 4000
EPS = 1e-5
SQD = math.sqrt(D)
ISQDK = 1.0 / math.sqrt(DK)
HV = H * (DK + 1)      # 780: natural V cols incl per-head ones column
KC = DCH * P           # 768
KVC = KC + HV          # 1548 bounce cols per token chunk

# packed per-layer bias/gain columns in `ball` [L, P, 78]
BQ, BK, BV, BO, B2, G1, BE1, G2, BE2, B1 = 0, 6, 12, 18, 24, 30, 36, 42, 48, 54

KV_GROUPS = [[0, 1, 2, 3], [4, 5, 6, 7]]
ALL_GROUP = [list(range(NCORE))]


def build(nc):
    def din(name, shape, dt=f32):
        return nc.dram_tensor(name, shape, dt, kind="ExternalInput").ap()

    tok = din("tok", [P, 2], i32)
    peTr = din("peTr", [P, DCH, TOK], bf16)
    embs = din("embs", [V, D], bf16)           # pre-scaled by sqrt(D)
    amask = din("amask", [2, 4, P, P], bf16)   # additive score masks
    wqkv = din("wqkv", [L, DCH, P, 3, D], bf16)
    wo_r = din("wo_r", [L, DCH, DK, 2, D], bf16)
    w1_r = din("w1_r", [L, 4, P, DCH, DCH * P], bf16)
    w2_r = din("w2_r", [L, FCH, P, D], bf16)
    ball = din("ball", [L, P, 78])
    gfp = din("gfp", [P, DCH])
    bfp = din("bfp", [P, DCH])
    woutc = din("woutc", [D, VPAD], bf16)
    boutp = din("boutp", [P, VCH])

    out = nc.dram_tensor("out", [VPAD, NCORE * TOK], f32,
                         kind="ExternalOutput").ap()

    kvins = [[nc.dram_tensor(f"kvin{l}_{t}", [P * KVC], bf16).ap()
              for t in range(2)] for l in range(L)]
    kvouts = [[nc.dram_tensor(f"kvout{l}_{t}", [GRP * P * KVC], bf16).ap()
               for t in range(2)] for l in range(L)]
    hinF = [nc.dram_tensor(f"hinF{t}", [P * KC], bf16).ap() for t in range(2)]
    houtF = [nc.dram_tensor(f"houtF{t}", [NCORE * P * KC], bf16,
                            addr_space="Shared").ap() for t in range(2)]

    with tile.TileContext(
            nc, trace_sim=os.environ.get("TRACE_SIM", "0") == "1",
    ) as tc, ExitStack() as octx, \
            nc.allow_low_precision(reason="bf16 datapath, fp32 accumulate"):
        const = octx.enter_context(tc.tile_pool(name="const", bufs=1))
        stats = octx.enter_context(tc.tile_pool(name="stats", bufs=10))
        # PSUM: 8 bank-slots total (every slot pads to a full 2KB bank):
        # bank(2) scores/logits, oT(1), ga(2) qkv/wo outs, yps(2), misc(1)
        psum = octx.enter_context(
            tc.tile_pool(name="psum", bufs=1, space="PSUM"))

        def ctile(shape, dt, nm):
            return const.tile(shape, dt, name=nm, tag=nm)

        ident_f = ctile([P, P], f32, "ident_f")
        make_identity(nc, ident_f[:])
        ident_b = ctile([P, P], bf16, "ident_b")
        nc.vector.tensor_copy(ident_b[:], ident_f[:])
        ones_col_b = ctile([P, 1], bf16, "ones_col_b")
        nc.vector.memset(ones_col_b[:], 1.0)
        ones_row_b = ctile([1, P], bf16, "ones_row_b")
        nc.vector.memset(ones_row_b[:], 1.0)
        zrow = ctile([1, 4 * P], bf16, "zrow")
        nc.vector.memset(zrow[:], 0.0)
        one_i = ctile([1, 1], i32, "one_i")
        nc.vector.memset(one_i[:], 1)
        magic_row = ctile([1, P], i32, "magic_row")
        nc.vector.memset(magic_row[:], 0x5F3759DF)
        eps_t = ctile([1, 1], f32, "eps_t")
        nc.vector.memset(eps_t[:], EPS)
        tokt = ctile([P, 2], i32, "tokt")
        nc.sync.dma_start(tokt[:], tok[:])
        mt = []
        for qi in range(2):
            row = []
            for s in range(4):
                m = ctile([P, P], bf16, f"mask{qi}_{s}")
                nc.sync.dma_start(m[:], amask[qi, s])
                row.append(m)
            mt.append(row)
        peTt = ctile([P, DCH, TOK], bf16, "peTt")
        nc.sync.dma_start(peTt[:], peTr[:])
        gft = ctile([P, DCH], f32, "gft")
        nc.sync.dma_start(gft[:], gfp[:])
        bft = ctile([P, DCH], f32, "bft")
        nc.sync.dma_start(bft[:], bfp[:])
        boutt = ctile([P, VCH], f32, "boutt")
        nc.sync.dma_start(boutt[:], boutp[:])

        with ExitStack() as lctx:
            acts = lctx.enter_context(tc.tile_pool(name="acts", bufs=34))
            sqp = lctx.enter_context(tc.tile_pool(name="sqp", bufs=4))
            bcp = lctx.enter_context(tc.tile_pool(name="bcp", bufs=4))
            qkvp = lctx.enter_context(tc.tile_pool(name="qkvp", bufs=6))
            vna = lctx.enter_context(tc.tile_pool(name="vna", bufs=3))
            ktp = lctx.enter_context(tc.tile_pool(name="ktp", bufs=9))
            vp = lctx.enter_context(tc.tile_pool(name="vp", bufs=9))
            ep = lctx.enter_context(tc.tile_pool(name="ep", bufs=3))
            hpp = lctx.enter_context(tc.tile_pool(name="hpp", bufs=8))
            ftp = lctx.enter_context(tc.tile_pool(name="ftp", bufs=6))
            wqp = lctx.enter_context(tc.tile_pool(name="wqp", bufs=7))
            wop = lctx.enter_context(tc.tile_pool(name="wop", bufs=7))
            w1p = lctx.enter_context(tc.tile_pool(name="w1p", bufs=4))
            w2p = lctx.enter_context(tc.tile_pool(name="w2p", bufs=6))
            bpool = lctx.enter_context(tc.tile_pool(name="bpool", bufs=3))

            ballts = {}

            def get_ball(l):
                if l not in ballts:
                    t = bpool.tile([P, 78], f32, name=f"ball{l}", tag="ball",
                                   bufs=3)
                    nc.sync.dma_start(t[:], ball[l])
                    ballts[l] = t
                return ballts[l]

            def load_qkv_w(l):
                wts = []
                for ic in range(DCH):
                    wt = wqp.tile([P, 3, D], bf16, name=f"wqkv{l}_{ic}",
                                  tag="wqkv", bufs=7)
                    nc.scalar.dma_start(wt[:], wqkv[l, ic])
                    wts.append(wt)
                return wts

            def _proj(pi, bcol, dst, xin, wts, ballt, tc0, tc1):
                gaA = psum.tile([P, 4, P], f32, name="gaA", tag="bank",
                                bufs=4)
                gaB = psum.tile([P, 4, P], f32, name="gaB", tag="bank",
                                bufs=4)
                outs = [gaA[:, oc, :] if oc < 4 else gaB[:, oc - 4, :]
                        for oc in range(DCH)]
                for oc in range(DCH):
                    for ic in range(DCH):
                        nc.tensor.matmul(
                            outs[oc],
                            lhsT=wts[ic][:, pi, oc * P:(oc + 1) * P],
                            rhs=xin[ic][:, tc0:tc1],
                            start=(ic == 0), stop=(ic == DCH - 1),
                            skip_group_check=True,
                        )
                for oc in range(DCH):
                    nc.vector.tensor_scalar_add(
                        dst[:, oc, tc0:tc1], outs[oc],
                        ballt[:, bcol + oc:bcol + oc + 1])

            def qkv_stage(l, tch, xin, wts, qTt, kTt, vTt):
                """Project x(tch) -> k,v first, bounce+AllGather, then q."""
                ballt = get_ball(l)
                tc0, tc1 = tch * P, (tch + 1) * P
                _proj(1, BK, kTt, xin, wts, ballt, tc0, tc1)
                _proj(2, BV, vTt, xin, wts, ballt, tc0, tc1)

                # natural V (+ ones cols) for this token chunk
                vn = vna.tile([P, HV], bf16, name=f"vn{l}_{tch}", tag="vn",
                              bufs=3)
                nc.gpsimd.memset(
                    vn[:].rearrange("p (h c) -> p h c", h=H)[:, :, DK:], 1.0)
                tpb = None
                for dd in range(DCH):
                    if dd % 4 == 0:
                        tpb = psum.tile([P, 4, P], bf16, name="vtp",
                                        tag="misc", bufs=1)
                    tp = tpb[:, dd % 4, :]
                    nc.tensor.transpose(tp, vTt[:, dd, tc0:tc1], ident_b[:])
                    for j in range(2):
                        h = 2 * dd + j
                        nc.scalar.activation(
                            vn[:, h * (DK + 1):h * (DK + 1) + DK],
                            tp[:, j * DK:(j + 1) * DK], AF.Identity)
                bounce_kv(l, tch, kTt, vn)
                _proj(0, BQ, qTt, xin, wts, ballt, tc0, tc1)

            def bounce_kv(l, tch, kTt, vn):
                kvi = kvins[l][tch].rearrange("(p c) -> p c", c=KVC)
                nc.sync.dma_start(
                    kvi[:, 0:KC].rearrange("p (d t) -> p d t", d=DCH),
                    kTt[:, :, tch * P:(tch + 1) * P])
                nc.sync.dma_start(kvi[:, KC:], vn[:])
                nc.gpsimd.collective_compute(
                    "AllGather", OP.bypass, replica_groups=KV_GROUPS,
                    ins=[kvins[l][tch].opt()], outs=[kvouts[l][tch].opt()])

            def load_kv(l, half):
                kvo = kvouts[l][half].rearrange("(g p c) -> g p c", p=P, c=KVC)
                KTs, Vs = [], []
                for g in range(GRP):
                    kt = ktp.tile([P, DCH, P], bf16, name=f"KT{half}_{g}",
                                  tag="kt", bufs=9)
                    nc.sync.dma_start(
                        kt[:], kvo[g, :, 0:KC].rearrange(
                            "p (d t) -> p d t", d=DCH))
                    KTs.append(kt)
                    v = vp.tile([P, HV], bf16, name=f"V{half}_{g}", tag="v",
                                bufs=9)
                    nc.sync.dma_start(v[:], kvo[g, :, KC:])
                    Vs.append(v)
                return KTs, Vs

            def attn_stage(qc, qTt, KT0, V0, KT1, V1):
                """Attention for query chunk qc (0: chunk r, 1: chunk 7-r).

                qc=0: keys = kvout0 slots (chunks 0-3), masked by mt[0].
                qc=1: keys = kvout0 (all visible) + kvout1 (mt[1] masks).
                Returns 6 hpT tiles [P, P] bf16 (head pairs x queries).
                """
                qc0, qc1_ = qc * P, (qc + 1) * P
                hpTs = []
                for hp in range(DCH):
                    oT2 = psum.tile([DK + 1, 2, P], f32, name="oT2", tag="oT",
                                    bufs=1)
                    oTs = []
                    for sub in range(2):
                        h = 2 * hp + sub
                        hr = sub * DK
                        banks = []
                        bank0 = psum.tile([P, 4 * P], f32, name="sc0",
                                          tag="bank", bufs=4)
                        if qc == 0:
                            for s in range(GRP):
                                nc.tensor.matmul(
                                    bank0[:, s * P:(s + 1) * P],
                                    lhsT=KT0[s][hr:hr + DK, hp, :],
                                    rhs=qTt[hr:hr + DK, hp, qc0:qc1_],
                                    start=True, stop=False,
                                    skip_group_check=True)
                                nc.tensor.matmul(
                                    bank0[:, s * P:(s + 1) * P],
                                    lhsT=ident_b[:],
                                    rhs=mt[0][s][:],
                                    start=False, stop=True,
                                    skip_group_check=True)
                            banks.append((bank0, V0))
                        else:
                            for s in range(GRP):
                                nc.tensor.matmul(
                                    bank0[:, s * P:(s + 1) * P],
                                    lhsT=KT0[s][hr:hr + DK, hp, :],
                                    rhs=qTt[hr:hr + DK, hp, qc0:qc1_],
                                    start=True, stop=True,
                                    skip_group_check=True)
                            banks.append((bank0, V0))
                            bank1 = psum.tile([P, 4 * P], f32, name="sc1",
                                              tag="bank", bufs=4)
                            for s in range(GRP):
                                nc.tensor.matmul(
                                    bank1[:, s * P:(s + 1) * P],
                                    lhsT=KT1[s][hr:hr + DK, hp, :],
                                    rhs=qTt[hr:hr + DK, hp, qc0:qc1_],
                                    start=True, stop=False,
                                    skip_group_check=True)
                                nc.tensor.matmul(
                                    bank1[:, s * P:(s + 1) * P],
                                    lhsT=ident_b[:],
                                    rhs=mt[1][s][:],
                                    start=False, stop=True,
                                    skip_group_check=True)
                            banks.append((bank1, V1))

                        oT = oT2[:, sub, :]
                        nbl = len(banks) * GRP
                        bi = 0
                        es = []
                        for bank, _vs in banks:
                            e = ep.tile([P, 4 * P], bf16, name="e", tag="e",
                                        bufs=3)
                            nc.scalar.activation(e[:], bank[:], AF.Exp,
                                                 scale=ISQDK)
                            es.append(e)
                        for (bank, vs), e in zip(banks, es):
                            for s in range(GRP):
                                nc.tensor.matmul(
                                    oT,
                                    lhsT=vs[s][:, h * (DK + 1):
                                               (h + 1) * (DK + 1)],
                                    rhs=e[:, s * P:(s + 1) * P],
                                    start=(bi == 0), stop=(bi == nbl - 1),
                                    skip_group_check=True)
                                bi += 1
                        oTs.append(oT)

                    den = stats.tile([1, 2, P], f32, name="den", tag="st")
                    nc.vector.tensor_copy(den[:, 0, :],
                                          oT2[DK:DK + 1, 0, :])
                    nc.vector.tensor_copy(den[:, 1, :],
                                          oT2[DK:DK + 1, 1, :])
                    rec = stats.tile([1, 2 * P], f32, name="rec", tag="st")
                    nc.vector.reciprocal_approx_fast(
                        rec[:], den[:].rearrange("a s p -> a (s p)"))
                    recb = stats.tile([1, 2 * P], bf16, name="recb", tag="st")
                    nc.vector.tensor_copy(recb[:], rec[:])
                    rb = psum.tile([DK, 2 * P], f32, name="rb", tag="misc",
                                   bufs=1)
                    nc.tensor.matmul(rb[:], lhsT=ones_row_b[:, 0:DK],
                                     rhs=recb[:], start=True, stop=True,
                                     skip_group_check=True)
                    rbs = bcp.tile([DK, 2 * P], bf16, name="rbs", tag="bc",
                                   bufs=4)
                    nc.vector.tensor_copy(rbs[:], rb[:])
                    for sub in range(2):
                        oh = hpp.tile([DK, P], bf16, name=f"oh{hp}_{sub}",
                                      tag="oh", bufs=14)
                        nc.vector.tensor_mul(oh[:], oT2[0:DK, sub, :],
                                             rbs[:, sub * P:(sub + 1) * P])
                        hpTs.append(oh)
                    hpTs.append(None)
                return hpTs

            def ln_stage(tch, tin, g_ap, gcol, be_ap, becol, tout):
                """LayerNorm over features for one token chunk.

                tin: 6 [P, TOK] bf16 tiles (reads [:, tch*P:+P]);
                tout: 6 tiles (writes same slice).
                """
                tc0, tc1 = tch * P, (tch + 1) * P
                stp = psum.tile([1, 2, P], f32, name="stp", tag="misc",
                                bufs=1)
                st_s = stp[:, 0, :]
                st_q = stp[:, 1, :]
                sqs = []
                for dd in range(DCH):
                    sq = sqp.tile([P, P], bf16, name="sq", tag="sq", bufs=6)
                    nc.scalar.activation(sq[:], tin[dd][:, tc0:tc1], AF.Square)
                    sqs.append(sq)
                for dd in range(DCH):
                    nc.tensor.matmul(st_s, lhsT=ones_col_b[:],
                                     rhs=tin[dd][:, tc0:tc1],
                                     start=(dd == 0), stop=(dd == DCH - 1),
                                     skip_group_check=True)
                for dd in range(DCH):
                    nc.tensor.matmul(st_q, lhsT=ones_col_b[:], rhs=sqs[dd][:],
                                     start=(dd == 0), stop=(dd == DCH - 1),
                                     skip_group_check=True)
                nm = stats.tile([1, P], bf16, name="nm", tag="st")
                nc.vector.tensor_scalar_mul(nm[:], st_s, -1.0 / D)
                m2 = stats.tile([1, P], f32, name="m2", tag="st")
                nc.vector.tensor_mul(m2[:], nm[:], nm[:])
                ex2 = stats.tile([1, P], f32, name="ex2", tag="st")
                nc.vector.tensor_scalar_mul(ex2[:], st_q, 1.0 / D)
                ve = stats.tile([1, P], f32, name="ve", tag="st")
                nc.vector.tensor_sub(ve[:], ex2[:], m2[:])
                nc.vector.tensor_scalar_add(ve[:], ve[:], EPS)
                vh = stats.tile([1, P], f32, name="vh", tag="st")
                nc.vector.tensor_scalar_mul(vh[:], ve[:], 0.5)
                yi = stats.tile([1, P], i32, name="yi", tag="st")
                nc.vector.tensor_scalar(yi[:], ve[:].bitcast(i32), one_i[:],
                                        None, op0=OP.arith_shift_right)
                nc.vector.tensor_sub(yi[:], magic_row[:], yi[:])
                y = yi[:].bitcast(f32)
                t = stats.tile([1, P], f32, name="t", tag="st")
                a = stats.tile([1, P], f32, name="a", tag="st")
                for _ in range(2):
                    nc.vector.tensor_mul(t[:], y, y)
                    nc.vector.tensor_mul(t[:], t[:], vh[:])
                    nc.vector.tensor_scalar(a[:], t[:], -1.0, 1.5,
                                            op0=OP.mult, op1=OP.add)
                    nc.vector.tensor_mul(y, a[:], y)
                rstd = stats.tile([1, P], bf16, name="rstd", tag="st")
                nc.vector.tensor_copy(rstd[:], y)
                bcps = psum.tile([P, 2, P], f32, name="bcps", tag="misc",
                                 bufs=1)
                nc.tensor.matmul(bcps[:, 0, :], lhsT=ones_row_b[:], rhs=nm[:],
                                 start=True, stop=True, skip_group_check=True)
                nc.tensor.matmul(bcps[:, 1, :], lhsT=ones_row_b[:],
                                 rhs=rstd[:],
                                 start=True, stop=True, skip_group_check=True)
                nmb = bcp.tile([P, P], bf16, name="nmbb", tag="bc", bufs=4)
                nc.vector.tensor_copy(nmb[:], bcps[:, 0, :])
                rsb = bcp.tile([P, P], bf16, name="rsbb", tag="bc", bufs=4)
                nc.vector.tensor_copy(rsb[:], bcps[:, 1, :])
                for dd in range(DCH):
                    eng = nc.vector if dd % 2 == 0 else nc.gpsimd
                    osl = tout[dd][:, tc0:tc1]
                    eng.tensor_add(osl, tin[dd][:, tc0:tc1], nmb[:])
                    eng.tensor_mul(osl, osl, rsb[:])
                    eng.tensor_scalar(
                        osl, osl, g_ap[:, gcol + dd:gcol + dd + 1],
                        be_ap[:, becol + dd:becol + dd + 1],
                        op0=OP.mult, op1=OP.add)

            def post_stage(l, tch, hpTs, x, t1, xn1, t2, xnext, wo_ts,
                           w1_cache, nxt):
                """wo+res+ln1+FFN+res+ln2 for one token chunk, then either
                qkv for layer l+1 (+ kv bounce/AG) or final LN + hT bounce."""
                ballt = get_ball(l)
                tc0, tc1 = tch * P, (tch + 1) * P
                # ---- Wo + residual ----
                gaA = psum.tile([P, 4, P], f32, name="woA", tag="bank",
                                bufs=4)
                gaB = psum.tile([P, 4, P], f32, name="woB", tag="bank",
                                bufs=4)
                ops_ = [gaA[:, oc, :] if oc < 4 else gaB[:, oc - 4, :]
                        for oc in range(DCH)]
                for oc in range(DCH):
                    for hp in range(DCH):
                        for sub in range(2):
                            oh = hpTs[hp * 3 + sub]
                            nc.tensor.matmul(
                                ops_[oc],
                                lhsT=wo_ts[hp][:, sub, oc * P:(oc + 1) * P],
                                rhs=oh[:],
                                start=(hp == 0 and sub == 0),
                                stop=(hp == DCH - 1 and sub == 1),
                                skip_group_check=True)
                for oc in range(DCH):
                    nc.vector.scalar_tensor_tensor(
                        t1[oc][:, tc0:tc1], ops_[oc],
                        ballt[:, BO + oc:BO + oc + 1], x[oc][:, tc0:tc1],
                        op0=OP.add, op1=OP.add)
                ln_stage(tch, t1, ballt, G1, ballt, BE1, xn1)
                # ---- FFN ----
                ypA = psum.tile([P, 4, P], f32, name="ypA", tag="yps",
                                bufs=2)
                ypB = psum.tile([P, 4, P], f32, name="ypB", tag="yps",
                                bufs=2)
                yps = [ypA[:, oc, :] if oc < 4 else ypB[:, oc - 4, :]
                       for oc in range(DCH)]
                for yp in (ypA, ypB):
                    nc.tensor.matmul(
                        yp[:].rearrange("p a b -> p (a b)"),
                        lhsT=ones_row_b[:], rhs=zrow[:],
                        start=True, stop=True, skip_group_check=True)
                fpb = None
                for og in range(4):
                    w1t = w1_cache[og]
                    for j in range(DCH):
                        fc = og * DCH + j
                        if fc % 2 == 0:
                            w2t = w2p.tile([P, 2, D], bf16, name="w2t",
                                           tag="w2", bufs=6)
                            nc.gpsimd.dma_start(w2t[:], w2_r[l, fc:fc + 2]
                                                .rearrange("f p d -> p f d"))
                        if fc % 4 == 0:
                            fpb = psum.tile([P, 4, P], f32, name="fpb",
                                            tag="misc", bufs=1)
                        fps = fpb[:, fc % 4, :]
                        for ic in range(DCH):
                            nc.tensor.matmul(
                                fps,
                                lhsT=w1t[:, ic, j * P:(j + 1) * P],
                                rhs=xn1[ic][:, tc0:tc1],
                                start=(ic == 0), stop=(ic == DCH - 1),
                                skip_group_check=True)
                        ft = ftp.tile([P, P], bf16, name="ft", tag="ft",
                                      bufs=6)
                        if fc % 2 == 0:
                            nc.vector.tensor_scalar(
                                ft[:], fps, ballt[:, B1 + fc:B1 + fc + 1],
                                0.0, op0=OP.add, op1=OP.max)
                        else:
                            nc.scalar.activation(
                                ft[:], fps, AF.Relu,
                                bias=ballt[:, B1 + fc:B1 + fc + 1])
                        for oc in range(DCH):
                            nc.tensor.matmul(
                                yps[oc],
                                lhsT=w2t[:, fc % 2, oc * P:(oc + 1) * P],
                                rhs=ft[:],
                                start=False, stop=(fc == FCH - 1),
                                skip_group_check=True)
                for oc in range(DCH):
                    nc.vector.scalar_tensor_tensor(
                        t2[oc][:, tc0:tc1], yps[oc],
                        ballt[:, B2 + oc:B2 + oc + 1], xn1[oc][:, tc0:tc1],
                        op0=OP.add, op1=OP.add)
                ln_stage(tch, t2, ballt, G2, ballt, BE2, xnext)
                # ---- next-layer qkv or final LN ----
                if l < L - 1:
                    qn, kn, vtn, wn = nxt
                    qkv_stage(l + 1, tch, xnext, wn, qn, kn, vtn)
                else:
                    hT = nxt
                    ln_stage(tch, xnext, gft, 0, bft, 0, hT)
                    hinr = hinF[tch].rearrange("(p c) -> p c", c=KC) \
                        .rearrange("p (d t) -> p d t", d=DCH)
                    for dd in range(DCH):
                        nc.sync.dma_start(hinr[:, dd, :],
                                          hT[dd][:, tc0:tc1])
                    nc.gpsimd.collective_compute(
                        "AllGather", OP.bypass, replica_groups=ALL_GROUP,
                        ins=[hinF[tch].opt()], outs=[houtF[tch].opt()])

            def xtiles(nm):
                return [acts.tile([P, TOK], bf16, name=f"{nm}{d}", tag="x",
                                  bufs=34) for d in range(DCH)]

            def qkvtiles(l):
                return [qkvp.tile([P, DCH, TOK], bf16, name=f"{nm}{l}",
                                  tag="qkv", bufs=6)
                        for nm in ("qT", "kT", "vT")]

            # ================= embedding + layer-0 qkv =================
            x = xtiles("x0_")
            w0 = load_qkv_w(0)
            qT, kT, vT = qkvtiles(0)
            for tch in range(2):
                g = sqp.tile([P, D], bf16, name="embrow", tag="emb", bufs=2)
                nc.gpsimd.indirect_dma_start(
                    out=g[:], out_offset=None, in_=embs[:],
                    in_offset=bass.IndirectOffsetOnAxis(
                        ap=tokt[:, tch:tch + 1], axis=0))
                tpb = None
                for dd in range(DCH):
                    if dd % 4 == 0:
                        tpb = psum.tile([P, 4, P], bf16, name="etp",
                                        tag="misc", bufs=1)
                    tp = tpb[:, dd % 4, :]
                    nc.tensor.transpose(tp, g[:, dd * P:(dd + 1) * P],
                                        ident_b[:])
                    nc.vector.tensor_add(
                        x[dd][:, tch * P:(tch + 1) * P], tp,
                        peTt[:, dd, tch * P:(tch + 1) * P])
                qkv_stage(0, tch, x, w0, qT, kT, vT)

            # ================= transformer layers =================
            for l in range(L):
                wo_ts = []
                for hp in range(DCH):
                    wt = wop.tile([DK, 2, D], bf16, name=f"wo{l}_{hp}",
                                  tag="wo", bufs=7)
                    nc.scalar.dma_start(wt[:], wo_r[l, hp])
                    wo_ts.append(wt)
                w1_cache = {}
                for og in range(4):
                    wt = w1p.tile([P, DCH, DCH * P], bf16,
                                  name=f"w1_{l}_{og}", tag="w1", bufs=4)
                    nc.scalar.dma_start(wt[:], w1_r[l, og])
                    w1_cache[og] = wt
                if l < L - 1:
                    wn = load_qkv_w(l + 1)
                    qn, kn, vtn = qkvtiles(l + 1)
                    nxt = (qn, kn, vtn, wn)
                else:
                    nxt = xtiles("hT_")
                t1, xn1, t2, xnext = (xtiles(f"t1_{l}"), xtiles(f"xn1_{l}"),
                                      xtiles(f"t2_{l}"), xtiles(f"x{l + 1}_"))

                KT0, V0 = load_kv(l, 0)
                KT1, V1 = load_kv(l, 1)
                with nc.named_scope(f"L{l}attA"):
                    hpA = attn_stage(0, qT, KT0, V0, None, None)
                with nc.named_scope(f"L{l}postA"):
                    post_stage(l, 0, hpA, x, t1, xn1, t2, xnext, wo_ts,
                               w1_cache, nxt)
                with nc.named_scope(f"L{l}attB"):
                    hpB = attn_stage(1, qT, KT0, V0, KT1, V1)
                with nc.named_scope(f"L{l}postB"):
                    post_stage(l, 1, hpB, x, t1, xn1, t2, xnext, wo_ts,
                               w1_cache, nxt)
                x = xnext
                if l < L - 1:
                    qT, kT, vT = nxt[0], nxt[1], nxt[2]

        # ================= vocab-parallel head =================
        with ExitStack() as hctx:
            htp = hctx.enter_context(tc.tile_pool(name="htp", bufs=12))
            wvp = hctx.enter_context(tc.tile_pool(name="wvp", bufs=7))
            otp = hctx.enter_context(tc.tile_pool(name="otp", bufs=3))

            HT = [[None] * DCH for _ in range(2)]
            for half in range(2):
                houtr = houtF[half].rearrange("(j p c) -> j p c", p=P, c=KC)
                for dd in range(DCH):
                    ht = htp.tile([P, NCORE * P], bf16,
                                  name=f"HT{half}_{dd}", tag="ht", bufs=12)
                    nc.sync.dma_start(
                        ht[:].rearrange("p (j t) -> p j t", j=NCORE),
                        houtr[:, :, dd * P:(dd + 1) * P]
                        .rearrange("j p t -> p j t"))
                    HT[half][dd] = ht

            for vh in range(2):
                wts = []
                for dd in range(DCH):
                    wt = wvp.tile([P, VPAD // 2], bf16, name="woutt",
                                  tag="wv", bufs=7)
                    nc.scalar.dma_start(
                        wt[:], woutc[dd * P:(dd + 1) * P,
                                     vh * (VPAD // 2):(vh + 1) * (VPAD // 2)])
                    wts.append(wt)
                for vc in range(VCH // 2):
                    vch = vh * (VCH // 2) + vc
                    ot = otp.tile([P, 2048], f32, name="lsb", tag="lsb",
                                  bufs=3)
                    qi = 0
                    for half in range(2):
                        for tq in range(2):
                            lp = psum.tile([P, 512], f32, name="logps",
                                           tag="bank", bufs=4)
                            for dd in range(DCH):
                                nc.tensor.matmul(
                                    lp[:],
                                    lhsT=wts[dd][:, vc * P:(vc + 1) * P],
                                    rhs=HT[half][dd][:, tq * 512:
                                                     (tq + 1) * 512],
                                    start=(dd == 0), stop=(dd == DCH - 1),
                                    skip_group_check=True)
                            osl = ot[:, half * 1024 + tq * 512:
                                     half * 1024 + (tq + 1) * 512]
                            if (qi + vc) % 2 == 0:
                                nc.scalar.activation(
                                    osl, lp[:], AF.Identity,
                                    bias=boutt[:, vch:vch + 1])
                            else:
                                nc.vector.tensor_scalar_add(
                                    osl, lp[:], boutt[:, vch:vch + 1])
                            qi += 1
                    eng = nc.sync if vch % 2 == 0 else nc.gpsimd
                    eng.dma_start(out[vch * P:(vch + 1) * P, :], ot[:])

    return nc


_CACHED = {}


def _compiled():
    if "nc" not in _CACHED:
        nc = bacc.Bacc("TRN2", target_bir_lowering=False, debug=False,
                       num_devices=NCORE)
        build(nc)
        nc.compile()
        _CACHED["nc"] = nc
    return _CACHED["nc"]


def _bf(a):
    return np.ascontiguousarray(np.asarray(a, np.float32)).astype(
        ml_dtypes.bfloat16)


def _make_inputs(tokens, emb, pe, wq, bq, wk, bk, wv, bv, wo, bo,
                 w1, b1, w2, b2, g1, be1, g2, be2, gf, bf, wout, bout):
    f = np.float32
    tokens = np.asarray(tokens).astype(np.int32)

    def parr(b):  # [L, dim] -> [L, P, dim//P]
        b = np.asarray(b, f)
        return b.reshape(L, b.shape[1] // P, P).transpose(0, 2, 1)

    def parr1(b):  # [dim] -> [P, dim//P]
        b = np.asarray(b, f)
        return np.ascontiguousarray(b.reshape(b.shape[0] // P, P).T)

    ball = np.concatenate(
        [parr(bq), parr(bk), parr(bv), parr(bo), parr(b2),
         parr(g1), parr(be1), parr(g2), parr(be2), parr(b1)], axis=2)

    # wqkv [L, DCH, P, 3, D]: lhsT chunks, natural rows
    wq_, wk_, wv_ = (np.asarray(w, f).reshape(L, DCH, P, D)
                     for w in (wq, wk, wv))
    wqkv = _bf(np.stack([wq_, wk_, wv_], axis=3))
    wo_r = _bf(np.asarray(wo, f).reshape(L, DCH, 2, DK, D)
               .transpose(0, 1, 3, 2, 4))
    # w1 [L, D, F] -> [L, 4, P(ic-row), DCH(ic), 768]
    w1_ = np.asarray(w1, f).reshape(L, DCH, P, 4, DCH * P)
    w1_r = _bf(w1_.transpose(0, 3, 2, 1, 4))
    w2_r = _bf(np.asarray(w2, f).reshape(L, FCH, P, D))

    emb_s = _bf(np.asarray(emb, f) * SQD)
    pe = np.asarray(pe, f)
    wout = np.asarray(wout, f)
    bout = np.asarray(bout, f)

    common = {
        "embs": emb_s, "wqkv": wqkv, "wo_r": wo_r, "w1_r": w1_r,
        "w2_r": w2_r, "ball": np.ascontiguousarray(ball),
        "gfp": parr1(gf), "bfp": parr1(bf),
    }

    tri = np.where(np.arange(P)[:, None] <= np.arange(P)[None, :],
                   0.0, -1e9).astype(f)  # [k, q]: visible iff k <= q
    zeros = np.zeros((P, P), f)
    neg = np.full((P, P), -1e9, f)

    in_maps = []
    for c in range(NCORE):
        b, r = divmod(c, GRP)
        chunks = (r, 7 - r)
        rows = np.concatenate(
            [np.arange(ch * P, (ch + 1) * P) for ch in chunks])
        tok_c = np.stack(
            [tokens[b, ch * P:(ch + 1) * P] for ch in chunks], axis=1
        ).astype(np.int32)
        peT_c = np.ascontiguousarray(pe[rows].T)  # [D, TOK]
        peTr = _bf(peT_c.reshape(DCH, P, TOK).transpose(1, 0, 2))

        # additive masks: amask[0][s] for qc1=chunk r vs key chunk s;
        # amask[1][g] for qc2=chunk 7-r vs key chunk 7-g.
        am = np.empty((2, 4, P, P), f)
        for s in range(4):
            am[0, s] = zeros if s < r else (tri if s == r else neg)
        for g2_ in range(4):
            am[1, g2_] = zeros if g2_ > r else (tri if g2_ == r else neg)

        wslice = np.zeros((D, VPAD), f)
        wslice[:, :VSH] = wout[:, c * VSH:(c + 1) * VSH]
        bslice = np.zeros((VPAD,), f)
        bslice[:VSH] = bout[c * VSH:(c + 1) * VSH]
        boutp_c = np.ascontiguousarray(bslice.reshape(VCH, P).T)

        m = dict(common)
        m.update({
            "tok": tok_c,
            "peTr": peTr,
            "amask": _bf(am),
            "woutc": _bf(wslice),
            "boutp": boutp_c,
        })
        in_maps.append(m)
    return in_maps


def run(in_maps, **kwargs):
    nc = _compiled()
    return run_bass_kernel_spmd(nc, in_maps, list(range(NCORE)), **kwargs)


def assemble(results):
    """results[c]['out'] [VPAD, 8*TOK] -> full logits [B, S, V].

    out col = half*1024 + j*128 + t, where half selects the token chunk
    (rank j owns chunks (j%4, 7-j%4)) and j is the source core.
    """
    full = np.empty((B, S, V), np.float32)
    for c in range(NCORE):
        lt = np.asarray(results[c]["out"])[:VSH]  # [4000, 2048]
        lg = lt.T  # [2048, 4000]
        for j in range(NCORE):
            bj, rj = divmod(j, GRP)
            for half, ch in enumerate((rj, 7 - rj)):
                full[bj, ch * P:(ch + 1) * P, c * VSH:(c + 1) * VSH] = \
                    lg[half * 1024 + j * P:half * 1024 + (j + 1) * P]
    return full


def kernel(**inputs):
    in_maps = _make_inputs(**inputs)
    res = run(in_maps)
    return assemble(res.results)


# revision 23
# speedup vs baseline: 1.1154x; 1.1154x over previous
"""MinimalGPT forward on 8 Trainium2 NeuronCores — v2.

Sharding: sequence-parallel transformer + vocab-parallel head (zigzag).
  core c: batch b=c//4, rank r=c%4, owns seq chunks (r, 7-r) = 2x128 tokens.

v2 changes vs v1:
  - bf16 weights + activations end-to-end (fp32 PSUM accumulate, fp32 LN
    stats); halves HBM + collective bytes, enables 128-wide matmuls at
    full PE rate.
  - causal structure: qc1 (chunk r) attends only to chunks 0-3, qc2 only
    to 0..7-r; additive masks applied as identity-matmuls into the score
    PSUM (uniform SPMD program, per-core mask data).
  - softmax denominators ride the AV matmul (ones column packed into V);
    reciprocal via DVE reciprocal_approx_fast on head-pairs.
  - per-layer kv AllGather split into two (one per token chunk), issued
    as soon as that chunk's kv is projected; attention over chunks 0-3
    starts after AG0, hiding most collective latency behind compute.
  - ACT engine uses a single fn table (exp/ln/square/identity); LN rstd
    computed as exp(-0.5*ln(var+eps)).
"""

import math
import os
import numpy as np
import ml_dtypes
from contextlib import ExitStack

import concourse.bass as bass
import concourse.tile as tile
from concourse import bacc, mybir
from concourse.bass_utils import run_bass_kernel_spmd
from concourse.masks import make_identity

f32 = mybir.dt.float32
bf16 = mybir.dt.bfloat16
i32 = mybir.dt.int32
AF = mybir.ActivationFunctionType
OP = mybir.AluOpType

V, D, H, L, F = 32000, 768, 12, 6, 3072
B, S = 2, 1024
P = 128
DK = 64
DCH = D // P           # 6
FCH = F // P           # 24
TOK = 256              # tokens per core (2 chunks of 128)
NCORE, GRP = 8, 4
VPAD = 4096
VCH = VPAD // P        # 32
VSH = V // NCORE       # 4000
EPS = 1e-5
SQD = math.sqrt(D)
ISQDK = 1.0 / math.sqrt(DK)
HV = H * (DK + 1)      # 780: natural V cols incl per-head ones column
KC = DCH * P           # 768
KVC = KC + HV          # 1548 bounce cols per token chunk

# packed per-layer bias/gain columns in `ball` [L, P, 78]
BQ, BK, BV, BO, B2, G1, BE1, G2, BE2, B1 = 0, 6, 12, 18, 24, 30, 36, 42, 48, 54

KV_GROUPS = [[0, 1, 2, 3], [4, 5, 6, 7]]
ALL_GROUP = [list(range(NCORE))]


def build(nc):
    def din(name, shape, dt=f32):
        return nc.dram_tensor(name, shape, dt, kind="ExternalInput").ap()

    tok = din("tok", [P, 2], i32)
    peTr = din("peTr", [P, DCH, TOK], bf16)
    embs = din("embs", [V, D], bf16)           # pre-scaled by sqrt(D)
    amask = din("amask", [2, 4, P, P], bf16)   # additive score masks
    wqkv = din("wqkv", [L, DCH, P, 3, D], bf16)
    wo_r = din("wo_r", [L, DCH, DK, 2, D], bf16)
    w1_r = din("w1_r", [L, 4, P, DCH, DCH * P], bf16)
    w2_r = din("w2_r", [L, FCH, P, D], bf16)
    ball = din("ball", [L, P, 78])
    gfp = din("gfp", [P, DCH])
    bfp = din("bfp", [P, DCH])
    woutc = din("woutc", [D, VPAD], bf16)
    boutp = din("boutp", [P, VCH])

    out = nc.dram_tensor("out", [VPAD, NCORE * TOK], f32,
                         kind="ExternalOutput").ap()

    kvins = [[nc.dram_tensor(f"kvin{l}_{t}", [P * KVC], bf16).ap()
              for t in range(2)] for l in range(L)]
    kvouts = [[nc.dram_tensor(f"kvout{l}_{t}", [GRP * P * KVC], bf16).ap()
               for t in range(2)] for l in range(L)]
    hinF = [nc.dram_tensor(f"hinF{t}", [P * KC], bf16).ap() for t in range(2)]
    houtF = [nc.dram_tensor(f"houtF{t}", [NCORE * P * KC], bf16,
                            addr_space="Shared").ap() for t in range(2)]

    with tile.TileContext(
            nc, trace_sim=os.environ.get("TRACE_SIM", "0") == "1",
    ) as tc, ExitStack() as octx, \
            nc.allow_low_precision(reason="bf16 datapath, fp32 accumulate"):
        const = octx.enter_context(tc.tile_pool(name="const", bufs=1))
        stats = octx.enter_context(tc.tile_pool(name="stats", bufs=10))
        # PSUM: 8 bank-slots total (every slot pads to a full 2KB bank):
        # bank(2) scores/logits, oT(1), ga(2) qkv/wo outs, yps(2), misc(1)
        psum = octx.enter_context(
            tc.tile_pool(name="psum", bufs=1, space="PSUM"))

        def ctile(shape, dt, nm):
            return const.tile(shape, dt, name=nm, tag=nm)

        ident_f = ctile([P, P], f32, "ident_f")
        make_identity(nc, ident_f[:])
        ident_b = ctile([P, P], bf16, "ident_b")
        nc.vector.tensor_copy(ident_b[:], ident_f[:])
        ones_col_b = ctile([P, 1], bf16, "ones_col_b")
        nc.vector.memset(ones_col_b[:], 1.0)
        ones_row_b = ctile([1, P], bf16, "ones_row_b")
        nc.vector.memset(ones_row_b[:], 1.0)
        zrow = ctile([1, 4 * P], bf16, "zrow")
        nc.vector.memset(zrow[:], 0.0)
        one_i = ctile([1, 1], i32, "one_i")
        nc.vector.memset(one_i[:], 1)
        magic_row = ctile([1, P], i32, "magic_row")
        nc.vector.memset(magic_row[:], 0x5F3759DF)
        eps_t = ctile([1, 1], f32, "eps_t")
        nc.vector.memset(eps_t[:], EPS)
        tokt = ctile([P, 2], i32, "tokt")
        nc.sync.dma_start(tokt[:], tok[:])
        mt = []
        for qi in range(2):
            row = []
            for s in range(4):
                m = ctile([P, P], bf16, f"mask{qi}_{s}")
                nc.sync.dma_start(m[:], amask[qi, s])
                row.append(m)
            mt.append(row)
        peTt = ctile([P, DCH, TOK], bf16, "peTt")
        nc.sync.dma_start(peTt[:], peTr[:])
        gft = ctile([P, DCH], f32, "gft")
        nc.sync.dma_start(gft[:], gfp[:])
        bft = ctile([P, DCH], f32, "bft")
        nc.sync.dma_start(bft[:], bfp[:])
        boutt = ctile([P, VCH], f32, "boutt")
        nc.sync.dma_start(boutt[:], boutp[:])

        with ExitStack() as lctx:
            acts = lctx.enter_context(tc.tile_pool(name="acts", bufs=34))
            sqp = lctx.enter_context(tc.tile_pool(name="sqp", bufs=4))
            bcp = lctx.enter_context(tc.tile_pool(name="bcp", bufs=4))
            qkvp = lctx.enter_context(tc.tile_pool(name="qkvp", bufs=6))
            vna = lctx.enter_context(tc.tile_pool(name="vna", bufs=3))
            ktp = lctx.enter_context(tc.tile_pool(name="ktp", bufs=9))
            vp = lctx.enter_context(tc.tile_pool(name="vp", bufs=9))
            ep = lctx.enter_context(tc.tile_pool(name="ep", bufs=3))
            hpp = lctx.enter_context(tc.tile_pool(name="hpp", bufs=8))
            ftp = lctx.enter_context(tc.tile_pool(name="ftp", bufs=6))
            wqp = lctx.enter_context(tc.tile_pool(name="wqp", bufs=7))
            wop = lctx.enter_context(tc.tile_pool(name="wop", bufs=7))
            w1p = lctx.enter_context(tc.tile_pool(name="w1p", bufs=4))
            w2p = lctx.enter_context(tc.tile_pool(name="w2p", bufs=6))
            bpool = lctx.enter_context(tc.tile_pool(name="bpool", bufs=3))

            ballts = {}

            def get_ball(l):
                if l not in ballts:
                    t = bpool.tile([P, 78], f32, name=f"ball{l}", tag="ball",
                                   bufs=3)
                    nc.sync.dma_start(t[:], ball[l])
                    ballts[l] = t
                return ballts[l]

            def load_qkv_w(l):
                wts = []
                for ic in range(DCH):
                    wt = wqp.tile([P, 3, D], bf16, name=f"wqkv{l}_{ic}",
                                  tag="wqkv", bufs=7)
                    nc.scalar.dma_start(wt[:], wqkv[l, ic])
                    wts.append(wt)
                return wts

            def _proj(pi, bcol, dst, xin, wts, ballt, tc0, tc1):
                gaA = psum.tile([P, 4, P], f32, name="gaA", tag="bank",
                                bufs=3)
                gaB = psum.tile([P, 4, P], f32, name="gaB", tag="bank",
                                bufs=3)
                outs = [gaA[:, oc, :] if oc < 4 else gaB[:, oc - 4, :]
                        for oc in range(DCH)]
                for oc in range(DCH):
                    for ic in range(DCH):
                        nc.tensor.matmul(
                            outs[oc],
                            lhsT=wts[ic][:, pi, oc * P:(oc + 1) * P],
                            rhs=xin[ic][:, tc0:tc1],
                            start=(ic == 0), stop=(ic == DCH - 1),
                            skip_group_check=True,
                        )
                for oc in range(DCH):
                    nc.vector.tensor_scalar_add(
                        dst[:, oc, tc0:tc1], outs[oc],
                        ballt[:, bcol + oc:bcol + oc + 1])

            def qkv_stage(l, tch, xin, wts, qTt, kTt, vTt):
                """Project x(tch) -> k,v first, bounce+AllGather, then q."""
                ballt = get_ball(l)
                tc0, tc1 = tch * P, (tch + 1) * P
                _proj(1, BK, kTt, xin, wts, ballt, tc0, tc1)
                _proj(2, BV, vTt, xin, wts, ballt, tc0, tc1)

                # natural V (+ ones cols) for this token chunk
                vn = vna.tile([P, HV], bf16, name=f"vn{l}_{tch}", tag="vn",
                              bufs=3)
                nc.gpsimd.memset(
                    vn[:].rearrange("p (h c) -> p h c", h=H)[:, :, DK:], 1.0)
                tpb = None
                for dd in range(DCH):
                    if dd % 4 == 0:
                        tpb = psum.tile([P, 4, P], bf16, name="vtp",
                                        tag="misc", bufs=1)
                    tp = tpb[:, dd % 4, :]
                    nc.tensor.transpose(tp, vTt[:, dd, tc0:tc1], ident_b[:])
                    for j in range(2):
                        h = 2 * dd + j
                        nc.scalar.activation(
                            vn[:, h * (DK + 1):h * (DK + 1) + DK],
                            tp[:, j * DK:(j + 1) * DK], AF.Identity)
                bounce_kv(l, tch, kTt, vn)
                _proj(0, BQ, qTt, xin, wts, ballt, tc0, tc1)

            def bounce_kv(l, tch, kTt, vn):
                kvi = kvins[l][tch].rearrange("(p c) -> p c", c=KVC)
                nc.sync.dma_start(
                    kvi[:, 0:KC].rearrange("p (d t) -> p d t", d=DCH),
                    kTt[:, :, tch * P:(tch + 1) * P])
                nc.sync.dma_start(kvi[:, KC:], vn[:])
                nc.gpsimd.collective_compute(
                    "AllGather", OP.bypass, replica_groups=KV_GROUPS,
                    ins=[kvins[l][tch].opt()], outs=[kvouts[l][tch].opt()])

            def load_kv(l, half):
                kvo = kvouts[l][half].rearrange("(g p c) -> g p c", p=P, c=KVC)
                KTs, Vs = [], []
                for g in range(GRP):
                    kt = ktp.tile([P, DCH, P], bf16, name=f"KT{half}_{g}",
                                  tag="kt", bufs=9)
                    nc.sync.dma_start(
                        kt[:], kvo[g, :, 0:KC].rearrange(
                            "p (d t) -> p d t", d=DCH))
                    KTs.append(kt)
                    v = vp.tile([P, HV], bf16, name=f"V{half}_{g}", tag="v",
                                bufs=9)
                    nc.sync.dma_start(v[:], kvo[g, :, KC:])
                    Vs.append(v)
                return KTs, Vs

            def attn_stage(qc, qTt, KT0, V0, KT1, V1):
                """Attention for query chunk qc (0: chunk r, 1: chunk 7-r).

                qc=0: keys = kvout0 slots (chunks 0-3), masked by mt[0].
                qc=1: keys = kvout0 (all visible) + kvout1 (mt[1] masks).
                Returns 6 hpT tiles [P, P] bf16 (head pairs x queries).
                """
                qc0, qc1_ = qc * P, (qc + 1) * P
                hpTs = []
                for hp in range(DCH):
                    aoT = psum.tile([P, 4, P], f32, name="aoT", tag="aoT",
                                    bufs=2)
                    oT2 = aoT[0:DK + 1, 0:2, :]
                    oTs = []
                    for sub in range(2):
                        h = 2 * hp + sub
                        hr = sub * DK
                        banks = []
                        bank0 = psum.tile([P, 4 * P], f32, name="sc0",
                                          tag="bank", bufs=3)
                        if qc == 0:
                            for s in range(GRP):
                                nc.tensor.matmul(
                                    bank0[:, s * P:(s + 1) * P],
                                    lhsT=KT0[s][hr:hr + DK, hp, :],
                                    rhs=qTt[hr:hr + DK, hp, qc0:qc1_],
                                    start=True, stop=False,
                                    skip_group_check=True)
                                nc.tensor.matmul(
                                    bank0[:, s * P:(s + 1) * P],
                                    lhsT=ident_b[:],
                                    rhs=mt[0][s][:],
                                    start=False, stop=True,
                                    skip_group_check=True)
                            banks.append((bank0, V0))
                        else:
                            for s in range(GRP):
                                nc.tensor.matmul(
                                    bank0[:, s * P:(s + 1) * P],
                                    lhsT=KT0[s][hr:hr + DK, hp, :],
                                    rhs=qTt[hr:hr + DK, hp, qc0:qc1_],
                                    start=True, stop=True,
                                    skip_group_check=True)
                            banks.append((bank0, V0))
                            bank1 = psum.tile([P, 4 * P], f32, name="sc1",
                                              tag="bank", bufs=3)
                            for s in range(GRP):
                                nc.tensor.matmul(
                                    bank1[:, s * P:(s + 1) * P],
                                    lhsT=KT1[s][hr:hr + DK, hp, :],
                                    rhs=qTt[hr:hr + DK, hp, qc0:qc1_],
                                    start=True, stop=False,
                                    skip_group_check=True)
                                nc.tensor.matmul(
                                    bank1[:, s * P:(s + 1) * P],
                                    lhsT=ident_b[:],
                                    rhs=mt[1][s][:],
                                    start=False, stop=True,
                                    skip_group_check=True)
                            banks.append((bank1, V1))

                        oT = oT2[:, sub, :]
                        nbl = len(banks) * GRP
                        bi = 0
                        es = []
                        for bank, _vs in banks:
                            e = ep.tile([P, 4 * P], bf16, name="e", tag="e",
                                        bufs=3)
                            nc.scalar.activation(e[:], bank[:], AF.Exp,
                                                 scale=ISQDK)
                            es.append(e)
                        for (bank, vs), e in zip(banks, es):
                            for s in range(GRP):
                                nc.tensor.matmul(
                                    oT,
                                    lhsT=vs[s][:, h * (DK + 1):
                                               (h + 1) * (DK + 1)],
                                    rhs=e[:, s * P:(s + 1) * P],
                                    start=(bi == 0), stop=(bi == nbl - 1),
                                    skip_group_check=True)
                                bi += 1
                        oTs.append(oT)

                    den = stats.tile([1, 2, P], f32, name="den", tag="st")
                    nc.vector.tensor_copy(den[:, 0, :],
                                          oT2[DK:DK + 1, 0, :])
                    nc.vector.tensor_copy(den[:, 1, :],
                                          oT2[DK:DK + 1, 1, :])
                    rec = stats.tile([1, 2 * P], f32, name="rec", tag="st")
                    nc.vector.reciprocal_approx_fast(
                        rec[:], den[:].rearrange("a s p -> a (s p)"))
                    recb = stats.tile([1, 2 * P], bf16, name="recb", tag="st")
                    nc.vector.tensor_copy(recb[:], rec[:])
                    rb = aoT[0:DK, 2:4, :]
                    nc.tensor.matmul(rb, lhsT=ones_row_b[:, 0:DK],
                                     rhs=recb[:], start=True, stop=True,
                                     skip_group_check=True)
                    rbs = bcp.tile([DK, 2 * P], bf16, name="rbs", tag="bc",
                                   bufs=4)
                    nc.vector.tensor_copy(
                        rbs[:], rb.rearrange("p a b -> p (a b)"))
                    for sub in range(2):
                        oh = hpp.tile([DK, P], bf16, name=f"oh{hp}_{sub}",
                                      tag="oh", bufs=14)
                        nc.vector.tensor_mul(oh[:], oT2[0:DK, sub, :],
                                             rbs[:, sub * P:(sub + 1) * P])
                        hpTs.append(oh)
                    hpTs.append(None)
                return hpTs

            def ln_stage(tch, tin, g_ap, gcol, be_ap, becol, tout):
                """LayerNorm over features for one token chunk.

                tin: 6 [P, TOK] bf16 tiles (reads [:, tch*P:+P]);
                tout: 6 tiles (writes same slice).
                """
                tc0, tc1 = tch * P, (tch + 1) * P
                stp = psum.tile([1, 2, P], f32, name="stp", tag="misc",
                                bufs=1)
                st_s = stp[:, 0, :]
                st_q = stp[:, 1, :]
                sqs = []
                for dd in range(DCH):
                    sq = sqp.tile([P, P], bf16, name="sq", tag="sq", bufs=6)
                    nc.scalar.activation(sq[:], tin[dd][:, tc0:tc1], AF.Square)
                    sqs.append(sq)
                for dd in range(DCH):
                    nc.tensor.matmul(st_s, lhsT=ones_col_b[:],
                                     rhs=tin[dd][:, tc0:tc1],
                                     start=(dd == 0), stop=(dd == DCH - 1),
                                     skip_group_check=True)
                for dd in range(DCH):
                    nc.tensor.matmul(st_q, lhsT=ones_col_b[:], rhs=sqs[dd][:],
                                     start=(dd == 0), stop=(dd == DCH - 1),
                                     skip_group_check=True)
                nm = stats.tile([1, P], bf16, name="nm", tag="st")
                nc.vector.tensor_scalar_mul(nm[:], st_s, -1.0 / D)
                m2 = stats.tile([1, P], f32, name="m2", tag="st")
                nc.vector.tensor_mul(m2[:], nm[:], nm[:])
                ex2 = stats.tile([1, P], f32, name="ex2", tag="st")
                nc.vector.tensor_scalar_mul(ex2[:], st_q, 1.0 / D)
                ve = stats.tile([1, P], f32, name="ve", tag="st")
                nc.vector.tensor_sub(ve[:], ex2[:], m2[:])
                nc.vector.tensor_scalar_add(ve[:], ve[:], EPS)
                vh = stats.tile([1, P], f32, name="vh", tag="st")
                nc.vector.tensor_scalar_mul(vh[:], ve[:], 0.5)
                yi = stats.tile([1, P], i32, name="yi", tag="st")
                nc.vector.tensor_scalar(yi[:], ve[:].bitcast(i32), one_i[:],
                                        None, op0=OP.arith_shift_right)
                nc.vector.tensor_sub(yi[:], magic_row[:], yi[:])
                y = yi[:].bitcast(f32)
                t = stats.tile([1, P], f32, name="t", tag="st")
                a = stats.tile([1, P], f32, name="a", tag="st")
                for _ in range(2):
                    nc.vector.tensor_mul(t[:], y, y)
                    nc.vector.tensor_mul(t[:], t[:], vh[:])
                    nc.vector.tensor_scalar(a[:], t[:], -1.0, 1.5,
                                            op0=OP.mult, op1=OP.add)
                    nc.vector.tensor_mul(y, a[:], y)
                rstd = stats.tile([1, P], bf16, name="rstd", tag="st")
                nc.vector.tensor_copy(rstd[:], y)
                bcps = psum.tile([P, 2, P], f32, name="bcps", tag="misc",
                                 bufs=1)
                nc.tensor.matmul(bcps[:, 0, :], lhsT=ones_row_b[:], rhs=nm[:],
                                 start=True, stop=True, skip_group_check=True)
                nc.tensor.matmul(bcps[:, 1, :], lhsT=ones_row_b[:],
                                 rhs=rstd[:],
                                 start=True, stop=True, skip_group_check=True)
                nmb = bcp.tile([P, P], bf16, name="nmbb", tag="bc", bufs=4)
                nc.vector.tensor_copy(nmb[:], bcps[:, 0, :])
                rsb = bcp.tile([P, P], bf16, name="rsbb", tag="bc", bufs=4)
                nc.vector.tensor_copy(rsb[:], bcps[:, 1, :])
                for dd in range(DCH):
                    eng = nc.vector if dd % 2 == 0 else nc.gpsimd
                    osl = tout[dd][:, tc0:tc1]
                    eng.tensor_add(osl, tin[dd][:, tc0:tc1], nmb[:])
                    eng.tensor_mul(osl, osl, rsb[:])
                    eng.tensor_scalar(
                        osl, osl, g_ap[:, gcol + dd:gcol + dd + 1],
                        be_ap[:, becol + dd:becol + dd + 1],
                        op0=OP.mult, op1=OP.add)

            def post_stage(l, tch, hpTs, x, t1, xn1, t2, xnext, wo_ts,
                           w1_cache, nxt):
                """wo+res+ln1+FFN+res+ln2 for one token chunk, then either
                qkv for layer l+1 (+ kv bounce/AG) or final LN + hT bounce."""
                ballt = get_ball(l)
                tc0, tc1 = tch * P, (tch + 1) * P
                # ---- Wo + residual ----
                gaA = psum.tile([P, 4, P], f32, name="woA", tag="bank",
                                bufs=3)
                gaB = psum.tile([P, 4, P], f32, name="woB", tag="bank",
                                bufs=3)
                ops_ = [gaA[:, oc, :] if oc < 4 else gaB[:, oc - 4, :]
                        for oc in range(DCH)]
                for oc in range(DCH):
                    for hp in range(DCH):
                        for sub in range(2):
                            oh = hpTs[hp * 3 + sub]
                            nc.tensor.matmul(
                                ops_[oc],
                                lhsT=wo_ts[hp][:, sub, oc * P:(oc + 1) * P],
                                rhs=oh[:],
                                start=(hp == 0 and sub == 0),
                                stop=(hp == DCH - 1 and sub == 1),
                                skip_group_check=True)
                for oc in range(DCH):
                    nc.vector.scalar_tensor_tensor(
                        t1[oc][:, tc0:tc1], ops_[oc],
                        ballt[:, BO + oc:BO + oc + 1], x[oc][:, tc0:tc1],
                        op0=OP.add, op1=OP.add)
                ln_stage(tch, t1, ballt, G1, ballt, BE1, xn1)
                # ---- FFN ----
                ypA = psum.tile([P, 4, P], f32, name="ypA", tag="yps",
                                bufs=2)
                ypB = psum.tile([P, 4, P], f32, name="ypB", tag="yps",
                                bufs=2)
                yps = [ypA[:, oc, :] if oc < 4 else ypB[:, oc - 4, :]
                       for oc in range(DCH)]
                for yp in (ypA, ypB):
                    nc.tensor.matmul(
                        yp[:].rearrange("p a b -> p (a b)"),
                        lhsT=ones_row_b[:], rhs=zrow[:],
                        start=True, stop=True, skip_group_check=True)
                fpb = None
                for og in range(4):
                    w1t = w1_cache[og]
                    for j in range(DCH):
                        fc = og * DCH + j
                        if fc % 2 == 0:
                            w2t = w2p.tile([P, 2, D], bf16, name="w2t",
                                           tag="w2", bufs=6)
                            nc.gpsimd.dma_start(w2t[:], w2_r[l, fc:fc + 2]
                                                .rearrange("f p d -> p f d"))
                        if fc % 4 == 0:
                            fpb = psum.tile([P, 4, P], f32, name="fpb",
                                            tag="bank", bufs=3)
                        fps = fpb[:, fc % 4, :]
                        for ic in range(DCH):
                            nc.tensor.matmul(
                                fps,
                                lhsT=w1t[:, ic, j * P:(j + 1) * P],
                                rhs=xn1[ic][:, tc0:tc1],
                                start=(ic == 0), stop=(ic == DCH - 1),
                                skip_group_check=True)
                        ft = ftp.tile([P, P], bf16, name="ft", tag="ft",
                                      bufs=6)
                        if fc % 2 == 0:
                            nc.vector.tensor_scalar(
                                ft[:], fps, ballt[:, B1 + fc:B1 + fc + 1],
                                0.0, op0=OP.add, op1=OP.max)
                        else:
                            nc.scalar.activation(
                                ft[:], fps, AF.Relu,
                                bias=ballt[:, B1 + fc:B1 + fc + 1])
                        for oc in range(DCH):
                            nc.tensor.matmul(
                                yps[oc],
                                lhsT=w2t[:, fc % 2, oc * P:(oc + 1) * P],
                                rhs=ft[:],
                                start=False, stop=(fc == FCH - 1),
                                skip_group_check=True)
                for oc in range(DCH):
                    nc.vector.scalar_tensor_tensor(
                        t2[oc][:, tc0:tc1], yps[oc],
                        ballt[:, B2 + oc:B2 + oc + 1], xn1[oc][:, tc0:tc1],
                        op0=OP.add, op1=OP.add)
                ln_stage(tch, t2, ballt, G2, ballt, BE2, xnext)
                # ---- next-layer qkv or final LN ----
                if l < L - 1:
                    qn, kn, vtn, wn = nxt
                    qkv_stage(l + 1, tch, xnext, wn, qn, kn, vtn)
                else:
                    hT = nxt
                    ln_stage(tch, xnext, gft, 0, bft, 0, hT)
                    hinr = hinF[tch].rearrange("(p c) -> p c", c=KC) \
                        .rearrange("p (d t) -> p d t", d=DCH)
                    for dd in range(DCH):
                        nc.sync.dma_start(hinr[:, dd, :],
                                          hT[dd][:, tc0:tc1])
                    nc.gpsimd.collective_compute(
                        "AllGather", OP.bypass, replica_groups=ALL_GROUP,
                        ins=[hinF[tch].opt()], outs=[houtF[tch].opt()])

            def xtiles(nm):
                return [acts.tile([P, TOK], bf16, name=f"{nm}{d}", tag="x",
                                  bufs=34) for d in range(DCH)]

            def qkvtiles(l):
                return [qkvp.tile([P, DCH, TOK], bf16, name=f"{nm}{l}",
                                  tag="qkv", bufs=6)
                        for nm in ("qT", "kT", "vT")]

            # ================= embedding + layer-0 qkv =================
            x = xtiles("x0_")
            w0 = load_qkv_w(0)
            qT, kT, vT = qkvtiles(0)
            for tch in range(2):
                g = sqp.tile([P, D], bf16, name="embrow", tag="emb", bufs=2)
                nc.gpsimd.indirect_dma_start(
                    out=g[:], out_offset=None, in_=embs[:],
                    in_offset=bass.IndirectOffsetOnAxis(
                        ap=tokt[:, tch:tch + 1], axis=0))
                tpb = None
                for dd in range(DCH):
                    if dd % 4 == 0:
                        tpb = psum.tile([P, 4, P], bf16, name="etp",
                                        tag="misc", bufs=1)
                    tp = tpb[:, dd % 4, :]
                    nc.tensor.transpose(tp, g[:, dd * P:(dd + 1) * P],
                                        ident_b[:])
                    nc.vector.tensor_add(
                        x[dd][:, tch * P:(tch + 1) * P], tp,
                        peTt[:, dd, tch * P:(tch + 1) * P])
                qkv_stage(0, tch, x, w0, qT, kT, vT)

            # ================= transformer layers =================
            for l in range(L):
                wo_ts = []
                for hp in range(DCH):
                    wt = wop.tile([DK, 2, D], bf16, name=f"wo{l}_{hp}",
                                  tag="wo", bufs=7)
                    nc.scalar.dma_start(wt[:], wo_r[l, hp])
                    wo_ts.append(wt)
                w1_cache = {}
                for og in range(4):
                    wt = w1p.tile([P, DCH, DCH * P], bf16,
                                  name=f"w1_{l}_{og}", tag="w1", bufs=4)
                    nc.scalar.dma_start(wt[:], w1_r[l, og])
                    w1_cache[og] = wt
                if l < L - 1:
                    wn = load_qkv_w(l + 1)
                    qn, kn, vtn = qkvtiles(l + 1)
                    nxt = (qn, kn, vtn, wn)
                else:
                    nxt = xtiles("hT_")
                t1, xn1, t2, xnext = (xtiles(f"t1_{l}"), xtiles(f"xn1_{l}"),
                                      xtiles(f"t2_{l}"), xtiles(f"x{l + 1}_"))

                KT0, V0 = load_kv(l, 0)
                KT1, V1 = load_kv(l, 1)
                with nc.named_scope(f"L{l}attA"):
                    hpA = attn_stage(0, qT, KT0, V0, None, None)
                with nc.named_scope(f"L{l}postA"):
                    post_stage(l, 0, hpA, x, t1, xn1, t2, xnext, wo_ts,
                               w1_cache, nxt)
                with nc.named_scope(f"L{l}attB"):
                    hpB = attn_stage(1, qT, KT0, V0, KT1, V1)
                with nc.named_scope(f"L{l}postB"):
                    post_stage(l, 1, hpB, x, t1, xn1, t2, xnext, wo_ts,
                               w1_cache, nxt)
                x = xnext
                if l < L - 1:
                    qT, kT, vT = nxt[0], nxt[1], nxt[2]

        # ================= vocab-parallel head =================
        with ExitStack() as hctx:
            htp = hctx.enter_context(tc.tile_pool(name="htp", bufs=12))
            wvp = hctx.enter_context(tc.tile_pool(name="wvp", bufs=7))
            otp = hctx.enter_context(tc.tile_pool(name="otp", bufs=3))

            HT = [[None] * DCH for _ in range(2)]
            for half in range(2):
                houtr = houtF[half].rearrange("(j p c) -> j p c", p=P, c=KC)
                for dd in range(DCH):
                    ht = htp.tile([P, NCORE * P], bf16,
                                  name=f"HT{half}_{dd}", tag="ht", bufs=12)
                    nc.sync.dma_start(
                        ht[:].rearrange("p (j t) -> p j t", j=NCORE),
                        houtr[:, :, dd * P:(dd + 1) * P]
                        .rearrange("j p t -> p j t"))
                    HT[half][dd] = ht

            for vh in range(2):
                wts = []
                for dd in range(DCH):
                    wt = wvp.tile([P, VPAD // 2], bf16, name="woutt",
                                  tag="wv", bufs=7)
                    nc.scalar.dma_start(
                        wt[:], woutc[dd * P:(dd + 1) * P,
                                     vh * (VPAD // 2):(vh + 1) * (VPAD // 2)])
                    wts.append(wt)
                for vc in range(VCH // 2):
                    vch = vh * (VCH // 2) + vc
                    ot = otp.tile([P, 2048], f32, name="lsb", tag="lsb",
                                  bufs=3)
                    qi = 0
                    for half in range(2):
                        for tq in range(2):
                            lp = psum.tile([P, 512], f32, name="logps",
                                           tag="bank", bufs=3)
                            for dd in range(DCH):
                                nc.tensor.matmul(
                                    lp[:],
                                    lhsT=wts[dd][:, vc * P:(vc + 1) * P],
                                    rhs=HT[half][dd][:, tq * 512:
                                                     (tq + 1) * 512],
                                    start=(dd == 0), stop=(dd == DCH - 1),
                                    skip_group_check=True)
                            osl = ot[:, half * 1024 + tq * 512:
                                     half * 1024 + (tq + 1) * 512]
                            if (qi + vc) % 2 == 0:
                                nc.scalar.activation(
                                    osl, lp[:], AF.Identity,
                                    bias=boutt[:, vch:vch + 1])
                            else:
                                nc.vector.tensor_scalar_add(
                                    osl, lp[:], boutt[:, vch:vch + 1])
                            qi += 1
                    eng = nc.sync if vch % 2 == 0 else nc.gpsimd
                    eng.dma_start(out[vch * P:(vch + 1) * P, :], ot[:])

    return nc


_CACHED = {}


def _compiled():
    if "nc" not in _CACHED:
        nc = bacc.Bacc("TRN2", target_bir_lowering=False, debug=False,
                       num_devices=NCORE)
        build(nc)
        nc.compile()
        _CACHED["nc"] = nc
    return _CACHED["nc"]


def _bf(a):
    return np.ascontiguousarray(np.asarray(a, np.float32)).astype(
        ml_dtypes.bfloat16)


def _make_inputs(tokens, emb, pe, wq, bq, wk, bk, wv, bv, wo, bo,
                 w1, b1, w2, b2, g1, be1, g2, be2, gf, bf, wout, bout):
    f = np.float32
    tokens = np.asarray(tokens).astype(np.int32)

    def parr(b):  # [L, dim] -> [L, P, dim//P]
        b = np.asarray(b, f)
        return b.reshape(L, b.shape[1] // P, P).transpose(0, 2, 1)

    def parr1(b):  # [dim] -> [P, dim//P]
        b = np.asarray(b, f)
        return np.ascontiguousarray(b.reshape(b.shape[0] // P, P).T)

    ball = np.concatenate(
        [parr(bq), parr(bk), parr(bv), parr(bo), parr(b2),
         parr(g1), parr(be1), parr(g2), parr(be2), parr(b1)], axis=2)

    # wqkv [L, DCH, P, 3, D]: lhsT chunks, natural rows
    wq_, wk_, wv_ = (np.asarray(w, f).reshape(L, DCH, P, D)
                     for w in (wq, wk, wv))
    wqkv = _bf(np.stack([wq_, wk_, wv_], axis=3))
    wo_r = _bf(np.asarray(wo, f).reshape(L, DCH, 2, DK, D)
               .transpose(0, 1, 3, 2, 4))
    # w1 [L, D, F] -> [L, 4, P(ic-row), DCH(ic), 768]
    w1_ = np.asarray(w1, f).reshape(L, DCH, P, 4, DCH * P)
    w1_r = _bf(w1_.transpose(0, 3, 2, 1, 4))
    w2_r = _bf(np.asarray(w2, f).reshape(L, FCH, P, D))

    emb_s = _bf(np.asarray(emb, f) * SQD)
    pe = np.asarray(pe, f)
    wout = np.asarray(wout, f)
    bout = np.asarray(bout, f)

    common = {
        "embs": emb_s, "wqkv": wqkv, "wo_r": wo_r, "w1_r": w1_r,
        "w2_r": w2_r, "ball": np.ascontiguousarray(ball),
        "gfp": parr1(gf), "bfp": parr1(bf),
    }

    tri = np.where(np.arange(P)[:, None] <= np.arange(P)[None, :],
                   0.0, -1e9).astype(f)  # [k, q]: visible iff k <= q
    zeros = np.zeros((P, P), f)
    neg = np.full((P, P), -1e9, f)

    in_maps = []
    for c in range(NCORE):
        b, r = divmod(c, GRP)
        chunks = (r, 7 - r)
        rows = np.concatenate(
            [np.arange(ch * P, (ch + 1) * P) for ch in chunks])
        tok_c = np.stack(
            [tokens[b, ch * P:(ch + 1) * P] for ch in chunks], axis=1
        ).astype(np.int32)
        peT_c = np.ascontiguousarray(pe[rows].T)  # [D, TOK]
        peTr = _bf(peT_c.reshape(DCH, P, TOK).transpose(1, 0, 2))

        # additive masks: amask[0][s] for qc1=chunk r vs key chunk s;
        # amask[1][g] for qc2=chunk 7-r vs key chunk 7-g.
        am = np.empty((2, 4, P, P), f)
        for s in range(4):
            am[0, s] = zeros if s < r else (tri if s == r else neg)
        for g2_ in range(4):
            am[1, g2_] = zeros if g2_ > r else (tri if g2_ == r else neg)

        wslice = np.zeros((D, VPAD), f)
        wslice[:, :VSH] = wout[:, c * VSH:(c + 1) * VSH]
        bslice = np.zeros((VPAD,), f)
        bslice[:VSH] = bout[c * VSH:(c + 1) * VSH]
        boutp_c = np.ascontiguousarray(bslice.reshape(VCH, P).T)

        m = dict(common)
        m.update({
            "tok": tok_c,
            "peTr": peTr,
            "amask": _bf(am),
            "woutc": _bf(wslice),
            "boutp": boutp_c,
        })
        in_maps.append(m)
    return in_maps


def run(in_maps, **kwargs):
    nc = _compiled()
    return run_bass_kernel_spmd(nc, in_maps, list(range(NCORE)), **kwargs)


def assemble(results):
    """results[c]['out'] [VPAD, 8*TOK] -> full logits [B, S, V].

    out col = half*1024 + j*128 + t, where half selects the token chunk
    (rank j owns chunks (j%4, 7-j%4)) and j is the source core.
    """
    full = np.empty((B, S, V), np.float32)
    for c in range(NCORE):
        lt = np.asarray(results[c]["out"])[:VSH]  # [4000, 2048]
        lg = lt.T  # [2048, 4000]
        for j in range(NCORE):
            bj, rj = divmod(j, GRP)
            for half, ch in enumerate((rj, 7 - rj)):
                full[bj, ch * P:(ch + 1) * P, c * VSH:(c + 1) * VSH] = \
                    lg[half * 1024 + j * P:half * 1024 + (j + 1) * P]
    return full


def kernel(**inputs):
    in_maps = _make_inputs(**inputs)
    res = run(in_maps)
    return assemble(res.results)


# revision 24
# speedup vs baseline: 1.1394x; 1.0216x over previous
"""MinimalGPT forward on 8 Trainium2 NeuronCores — v2.

Sharding: sequence-parallel transformer + vocab-parallel head (zigzag).
  core c: batch b=c//4, rank r=c%4, owns seq chunks (r, 7-r) = 2x128 tokens.

v2 changes vs v1:
  - bf16 weights + activations end-to-end (fp32 PSUM accumulate, fp32 LN
    stats); halves HBM + collective bytes, enables 128-wide matmuls at
    full PE rate.
  - causal structure: qc1 (chunk r) attends only to chunks 0-3, qc2 only
    to 0..7-r; additive masks applied as identity-matmuls into the score
    PSUM (uniform SPMD program, per-core mask data).
  - softmax denominators ride the AV matmul (ones column packed into V);
    reciprocal via DVE reciprocal_approx_fast on head-pairs.
  - per-layer kv AllGather split into two (one per token chunk), issued
    as soon as that chunk's kv is projected; attention over chunks 0-3
    starts after AG0, hiding most collective latency behind compute.
  - ACT engine uses a single fn table (exp/ln/square/identity); LN rstd
    computed as exp(-0.5*ln(var+eps)).
"""

import math
import os
import numpy as np
import ml_dtypes
from contextlib import ExitStack

import concourse.bass as bass
import concourse.tile as tile
from concourse import bacc, mybir
from concourse.bass_utils import run_bass_kernel_spmd
from concourse.masks import make_identity

f32 = mybir.dt.float32
bf16 = mybir.dt.bfloat16
i32 = mybir.dt.int32
AF = mybir.ActivationFunctionType
OP = mybir.AluOpType

V, D, H, L, F = 32000, 768, 12, 6, 3072
B, S = 2, 1024
P = 128
DK = 64
DCH = D // P           # 6
FCH = F // P           # 24
TOK = 256              # tokens per core (2 chunks of 128)
NCORE, GRP = 8, 4
VPAD = 4096
VCH = VPAD // P        # 32
VSH = V // NCORE       # 4000
EPS = 1e-5
SQD = math.sqrt(D)
ISQDK = 1.0 / math.sqrt(DK)
HV = H * (DK + 1)      # 780: natural V cols incl per-head ones column
KC = DCH * P           # 768
KVC = KC + HV          # 1548 bounce cols per token chunk

# packed per-layer bias/gain columns in `ball` [L, P, 78]
BQ, BK, BV, BO, B2, G1, BE1, G2, BE2, B1 = 0, 6, 12, 18, 24, 30, 36, 42, 48, 54

KV_GROUPS = [[0, 1, 2, 3], [4, 5, 6, 7]]
ALL_GROUP = [list(range(NCORE))]


def build(nc):
    def din(name, shape, dt=f32):
        return nc.dram_tensor(name, shape, dt, kind="ExternalInput").ap()

    tok = din("tok", [P, 2], i32)
    peTr = din("peTr", [P, DCH, TOK], bf16)
    embs = din("embs", [V, D], bf16)           # pre-scaled by sqrt(D)
    amask = din("amask", [2, 4, P, P], bf16)   # additive score masks
    wqkv = din("wqkv", [L, DCH, P, 3, D], bf16)
    wo_r = din("wo_r", [L, DCH, DK, 2, D], bf16)
    w1_r = din("w1_r", [L, 4, P, DCH, DCH * P], bf16)
    w2_r = din("w2_r", [L, FCH, P, D], bf16)
    ball = din("ball", [L, P, 78])
    gfp = din("gfp", [P, DCH])
    bfp = din("bfp", [P, DCH])
    woutc = din("woutc", [D, VPAD], bf16)
    boutp = din("boutp", [P, VCH])

    out = nc.dram_tensor("out", [VPAD, NCORE * TOK], f32,
                         kind="ExternalOutput").ap()

    kvins = [[nc.dram_tensor(f"kvin{l}_{t}", [P * KVC], bf16).ap()
              for t in range(2)] for l in range(L)]
    kvouts = [[nc.dram_tensor(f"kvout{l}_{t}", [GRP * P * KVC], bf16).ap()
               for t in range(2)] for l in range(L)]
    hinF = [nc.dram_tensor(f"hinF{t}", [P * KC], bf16).ap() for t in range(2)]
    houtF = [nc.dram_tensor(f"houtF{t}", [NCORE * P * KC], bf16,
                            addr_space="Shared").ap() for t in range(2)]

    with tile.TileContext(
            nc, trace_sim=os.environ.get("TRACE_SIM", "0") == "1",
    ) as tc, ExitStack() as octx, \
            nc.allow_low_precision(reason="bf16 datapath, fp32 accumulate"):
        const = octx.enter_context(tc.tile_pool(name="const", bufs=1))
        stats = octx.enter_context(tc.tile_pool(name="stats", bufs=10))
        # PSUM: 8 bank-slots total (every slot pads to a full 2KB bank):
        # bank(2) scores/logits, oT(1), ga(2) qkv/wo outs, yps(2), misc(1)
        psum = octx.enter_context(
            tc.tile_pool(name="psum", bufs=1, space="PSUM"))

        def ctile(shape, dt, nm):
            return const.tile(shape, dt, name=nm, tag=nm)

        ident_f = ctile([P, P], f32, "ident_f")
        make_identity(nc, ident_f[:])
        ident_b = ctile([P, P], bf16, "ident_b")
        nc.vector.tensor_copy(ident_b[:], ident_f[:])
        ones_col_b = ctile([P, 1], bf16, "ones_col_b")
        nc.vector.memset(ones_col_b[:], 1.0)
        ones_row_b = ctile([1, P], bf16, "ones_row_b")
        nc.vector.memset(ones_row_b[:], 1.0)
        zrow = ctile([1, 4 * P], bf16, "zrow")
        nc.vector.memset(zrow[:], 0.0)
        one_i = ctile([1, 1], i32, "one_i")
        nc.vector.memset(one_i[:], 1)
        magic_row = ctile([1, P], i32, "magic_row")
        nc.vector.memset(magic_row[:], 0x5F3759DF)
        eps_t = ctile([1, 1], f32, "eps_t")
        nc.vector.memset(eps_t[:], EPS)
        tokt = ctile([P, 2], i32, "tokt")
        nc.sync.dma_start(tokt[:], tok[:])
        mt = []
        for qi in range(2):
            row = []
            for s in range(4):
                m = ctile([P, P], bf16, f"mask{qi}_{s}")
                nc.sync.dma_start(m[:], amask[qi, s])
                row.append(m)
            mt.append(row)
        peTt = ctile([P, DCH, TOK], bf16, "peTt")
        nc.sync.dma_start(peTt[:], peTr[:])
        gft = ctile([P, DCH], f32, "gft")
        nc.sync.dma_start(gft[:], gfp[:])
        bft = ctile([P, DCH], f32, "bft")
        nc.sync.dma_start(bft[:], bfp[:])
        boutt = ctile([P, VCH], f32, "boutt")
        nc.sync.dma_start(boutt[:], boutp[:])

        with ExitStack() as lctx:
            acts = lctx.enter_context(tc.tile_pool(name="acts", bufs=34))
            sqp = lctx.enter_context(tc.tile_pool(name="sqp", bufs=4))
            bcp = lctx.enter_context(tc.tile_pool(name="bcp", bufs=4))
            qkvp = lctx.enter_context(tc.tile_pool(name="qkvp", bufs=6))
            vna = lctx.enter_context(tc.tile_pool(name="vna", bufs=3))
            ktp = lctx.enter_context(tc.tile_pool(name="ktp", bufs=9))
            vp = lctx.enter_context(tc.tile_pool(name="vp", bufs=9))
            ep = lctx.enter_context(tc.tile_pool(name="ep", bufs=3))
            hpp = lctx.enter_context(tc.tile_pool(name="hpp", bufs=8))
            ftp = lctx.enter_context(tc.tile_pool(name="ftp", bufs=6))
            wqp = lctx.enter_context(tc.tile_pool(name="wqp", bufs=7))
            wop = lctx.enter_context(tc.tile_pool(name="wop", bufs=7))
            w1p = lctx.enter_context(tc.tile_pool(name="w1p", bufs=4))
            w2p = lctx.enter_context(tc.tile_pool(name="w2p", bufs=6))
            bpool = lctx.enter_context(tc.tile_pool(name="bpool", bufs=3))

            ballts = {}

            def get_ball(l):
                if l not in ballts:
                    t = bpool.tile([P, 78], f32, name=f"ball{l}", tag="ball",
                                   bufs=3)
                    nc.sync.dma_start(t[:], ball[l])
                    ballts[l] = t
                return ballts[l]

            def load_qkv_w(l):
                wts = []
                for ic in range(DCH):
                    wt = wqp.tile([P, 3, D], bf16, name=f"wqkv{l}_{ic}",
                                  tag="wqkv", bufs=7)
                    nc.scalar.dma_start(wt[:], wqkv[l, ic])
                    wts.append(wt)
                return wts

            def _proj(pi, bcol, dst, xin, wts, ballt, tc0, tc1):
                gaA = psum.tile([P, 4, P], f32, name="gaA", tag="bank",
                                bufs=3)
                gaB = psum.tile([P, 4, P], f32, name="gaB", tag="bank",
                                bufs=3)
                outs = [gaA[:, oc, :] if oc < 4 else gaB[:, oc - 4, :]
                        for oc in range(DCH)]
                for oc in range(DCH):
                    for ic in range(DCH):
                        nc.tensor.matmul(
                            outs[oc],
                            lhsT=wts[ic][:, pi, oc * P:(oc + 1) * P],
                            rhs=xin[ic][:, tc0:tc1],
                            start=(ic == 0), stop=(ic == DCH - 1),
                            skip_group_check=True,
                        )
                for oc in range(DCH):
                    nc.vector.tensor_scalar_add(
                        dst[:, oc, tc0:tc1], outs[oc],
                        ballt[:, bcol + oc:bcol + oc + 1])

            def qkv_stage(l, tch, xin, wts, qTt, kTt, vTt):
                """Project x(tch) -> k,v first, bounce+AllGather, then q."""
                ballt = get_ball(l)
                tc0, tc1 = tch * P, (tch + 1) * P
                _proj(1, BK, kTt, xin, wts, ballt, tc0, tc1)
                _proj(2, BV, vTt, xin, wts, ballt, tc0, tc1)

                # natural V (+ ones cols) for this token chunk
                vn = vna.tile([P, HV], bf16, name=f"vn{l}_{tch}", tag="vn",
                              bufs=3)
                nc.gpsimd.memset(
                    vn[:].rearrange("p (h c) -> p h c", h=H)[:, :, DK:], 1.0)
                tpb = None
                for dd in range(DCH):
                    if dd % 4 == 0:
                        tpb = psum.tile([P, 4, P], bf16, name="vtp",
                                        tag="misc", bufs=1)
                    tp = tpb[:, dd % 4, :]
                    nc.tensor.transpose(tp, vTt[:, dd, tc0:tc1], ident_b[:])
                    for j in range(2):
                        h = 2 * dd + j
                        nc.scalar.activation(
                            vn[:, h * (DK + 1):h * (DK + 1) + DK],
                            tp[:, j * DK:(j + 1) * DK], AF.Identity)
                bounce_kv(l, tch, kTt, vn)
                _proj(0, BQ, qTt, xin, wts, ballt, tc0, tc1)

            def bounce_kv(l, tch, kTt, vn):
                kvi = kvins[l][tch].rearrange("(p c) -> p c", c=KVC)
                nc.sync.dma_start(
                    kvi[:, 0:KC].rearrange("p (d t) -> p d t", d=DCH),
                    kTt[:, :, tch * P:(tch + 1) * P])
                nc.sync.dma_start(kvi[:, KC:], vn[:])
                nc.gpsimd.collective_compute(
                    "AllGather", OP.bypass, replica_groups=KV_GROUPS,
                    ins=[kvins[l][tch].opt()], outs=[kvouts[l][tch].opt()])

            def load_kv(l, half):
                kvo = kvouts[l][half].rearrange("(g p c) -> g p c", p=P, c=KVC)
                KTs, Vs = [], []
                for g in range(GRP):
                    kt = ktp.tile([P, DCH, P], bf16, name=f"KT{half}_{g}",
                                  tag="kt", bufs=9)
                    nc.sync.dma_start(
                        kt[:], kvo[g, :, 0:KC].rearrange(
                            "p (d t) -> p d t", d=DCH))
                    KTs.append(kt)
                    v = vp.tile([P, HV], bf16, name=f"V{half}_{g}", tag="v",
                                bufs=9)
                    nc.sync.dma_start(v[:], kvo[g, :, KC:])
                    Vs.append(v)
                return KTs, Vs

            def attn_stage(qc, qTt, KT0, V0, KT1, V1):
                """Attention for query chunk qc, software-pipelined: the
                score matmuls of unit i+1 are emitted before the AV matmuls
                of unit i so the PE never waits on the exp (ACT) feedback."""
                qc0, qc1_ = qc * P, (qc + 1) * P
                hpTs = []
                aoTs = {}

                def build(hp, sub):
                    h = 2 * hp + sub
                    hr = sub * DK
                    banks = []
                    bank0 = psum.tile([P, 4 * P], f32, name="sc0",
                                      tag="bank", bufs=3)
                    if qc == 0:
                        for s in range(GRP):
                            nc.tensor.matmul(
                                bank0[:, s * P:(s + 1) * P],
                                lhsT=KT0[s][hr:hr + DK, hp, :],
                                rhs=qTt[hr:hr + DK, hp, qc0:qc1_],
                                start=True, stop=False,
                                skip_group_check=True)
                            nc.tensor.matmul(
                                bank0[:, s * P:(s + 1) * P],
                                lhsT=ident_b[:],
                                rhs=mt[0][s][:],
                                start=False, stop=True,
                                skip_group_check=True)
                        banks.append((bank0, V0))
                    else:
                        for s in range(GRP):
                            nc.tensor.matmul(
                                bank0[:, s * P:(s + 1) * P],
                                lhsT=KT0[s][hr:hr + DK, hp, :],
                                rhs=qTt[hr:hr + DK, hp, qc0:qc1_],
                                start=True, stop=True,
                                skip_group_check=True)
                        banks.append((bank0, V0))
                        bank1 = psum.tile([P, 4 * P], f32, name="sc1",
                                          tag="bank", bufs=3)
                        for s in range(GRP):
                            nc.tensor.matmul(
                                bank1[:, s * P:(s + 1) * P],
                                lhsT=KT1[s][hr:hr + DK, hp, :],
                                rhs=qTt[hr:hr + DK, hp, qc0:qc1_],
                                start=True, stop=False,
                                skip_group_check=True)
                            nc.tensor.matmul(
                                bank1[:, s * P:(s + 1) * P],
                                lhsT=ident_b[:],
                                rhs=mt[1][s][:],
                                start=False, stop=True,
                                skip_group_check=True)
                        banks.append((bank1, V1))
                    es = []
                    for bank, _vs in banks:
                        e = ep.tile([P, 4 * P], bf16, name="e", tag="e",
                                    bufs=3)
                        nc.scalar.activation(e[:], bank[:], AF.Exp,
                                             scale=ISQDK)
                        es.append(e)
                    return (hp, sub, banks, es)

                def do_av(item):
                    hp, sub, banks, es = item
                    h = 2 * hp + sub
                    if sub == 0:
                        aoTs[hp] = psum.tile([P, 4, P], f32, name="aoT",
                                             tag="aoT", bufs=2)
                    oT = aoTs[hp][0:DK + 1, sub, :]
                    nbl = len(banks) * GRP
                    bi = 0
                    for (bank, vs), e in zip(banks, es):
                        for s in range(GRP):
                            nc.tensor.matmul(
                                oT,
                                lhsT=vs[s][:, h * (DK + 1):
                                           (h + 1) * (DK + 1)],
                                rhs=e[:, s * P:(s + 1) * P],
                                start=(bi == 0), stop=(bi == nbl - 1),
                                skip_group_check=True)
                            bi += 1

                def finish(hp):
                    aoT = aoTs.pop(hp)
                    oT2 = aoT[0:DK + 1, 0:2, :]
                    den = stats.tile([1, 2, P], f32, name="den", tag="st")
                    nc.vector.tensor_copy(den[:, 0, :],
                                          oT2[DK:DK + 1, 0, :])
                    nc.vector.tensor_copy(den[:, 1, :],
                                          oT2[DK:DK + 1, 1, :])
                    rec = stats.tile([1, 2 * P], f32, name="rec", tag="st")
                    nc.vector.reciprocal_approx_fast(
                        rec[:], den[:].rearrange("a s p -> a (s p)"))
                    recb = stats.tile([1, 2 * P], bf16, name="recb",
                                      tag="st")
                    nc.vector.tensor_copy(recb[:], rec[:])
                    rb = aoT[0:DK, 2:4, :]
                    nc.tensor.matmul(rb, lhsT=ones_row_b[:, 0:DK],
                                     rhs=recb[:], start=True, stop=True,
                                     skip_group_check=True)
                    rbs = bcp.tile([DK, 2 * P], bf16, name="rbs", tag="bc",
                                   bufs=4)
                    nc.vector.tensor_copy(
                        rbs[:], rb.rearrange("p a b -> p (a b)"))
                    for sub in range(2):
                        oh = hpp.tile([DK, P], bf16, name=f"oh{hp}_{sub}",
                                      tag="oh", bufs=14)
                        nc.vector.tensor_mul(oh[:], oT2[0:DK, sub, :],
                                             rbs[:, sub * P:(sub + 1) * P])
                        hpTs.append(oh)
                    hpTs.append(None)

                prev = None
                for hp in range(DCH):
                    for sub in range(2):
                        item = build(hp, sub)
                        if prev is not None:
                            do_av(prev)
                            if prev[1] == 1:
                                finish(prev[0])
                        prev = item
                do_av(prev)
                finish(prev[0])
                return hpTs

            def ln_stage(tch, tin, g_ap, gcol, be_ap, becol, tout):
                """LayerNorm over features for one token chunk.

                tin: 6 [P, TOK] bf16 tiles (reads [:, tch*P:+P]);
                tout: 6 tiles (writes same slice).
                """
                tc0, tc1 = tch * P, (tch + 1) * P
                stp = psum.tile([1, 2, P], f32, name="stp", tag="misc",
                                bufs=1)
                st_s = stp[:, 0, :]
                st_q = stp[:, 1, :]
                sqs = []
                for dd in range(DCH):
                    sq = sqp.tile([P, P], bf16, name="sq", tag="sq", bufs=6)
                    nc.scalar.activation(sq[:], tin[dd][:, tc0:tc1], AF.Square)
                    sqs.append(sq)
                for dd in range(DCH):
                    nc.tensor.matmul(st_s, lhsT=ones_col_b[:],
                                     rhs=tin[dd][:, tc0:tc1],
                                     start=(dd == 0), stop=(dd == DCH - 1),
                                     skip_group_check=True)
                for dd in range(DCH):
                    nc.tensor.matmul(st_q, lhsT=ones_col_b[:], rhs=sqs[dd][:],
                                     start=(dd == 0), stop=(dd == DCH - 1),
                                     skip_group_check=True)
                nm = stats.tile([1, P], bf16, name="nm", tag="st")
                nc.vector.tensor_scalar_mul(nm[:], st_s, -1.0 / D)
                m2 = stats.tile([1, P], f32, name="m2", tag="st")
                nc.vector.tensor_mul(m2[:], nm[:], nm[:])
                ex2 = stats.tile([1, P], f32, name="ex2", tag="st")
                nc.vector.tensor_scalar_mul(ex2[:], st_q, 1.0 / D)
                ve = stats.tile([1, P], f32, name="ve", tag="st")
                nc.vector.tensor_sub(ve[:], ex2[:], m2[:])
                nc.vector.tensor_scalar_add(ve[:], ve[:], EPS)
                vh = stats.tile([1, P], f32, name="vh", tag="st")
                nc.vector.tensor_scalar_mul(vh[:], ve[:], 0.5)
                yi = stats.tile([1, P], i32, name="yi", tag="st")
                nc.vector.tensor_scalar(yi[:], ve[:].bitcast(i32), one_i[:],
                                        None, op0=OP.arith_shift_right)
                nc.vector.tensor_sub(yi[:], magic_row[:], yi[:])
                y = yi[:].bitcast(f32)
                t = stats.tile([1, P], f32, name="t", tag="st")
                a = stats.tile([1, P], f32, name="a", tag="st")
                for _ in range(2):
                    nc.vector.tensor_mul(t[:], y, y)
                    nc.vector.tensor_mul(t[:], t[:], vh[:])
                    nc.vector.tensor_scalar(a[:], t[:], -1.0, 1.5,
                                            op0=OP.mult, op1=OP.add)
                    nc.vector.tensor_mul(y, a[:], y)
                rstd = stats.tile([1, P], bf16, name="rstd", tag="st")
                nc.vector.tensor_copy(rstd[:], y)
                bcps = psum.tile([P, 2, P], f32, name="bcps", tag="misc",
                                 bufs=1)
                nc.tensor.matmul(bcps[:, 0, :], lhsT=ones_row_b[:], rhs=nm[:],
                                 start=True, stop=True, skip_group_check=True)
                nc.tensor.matmul(bcps[:, 1, :], lhsT=ones_row_b[:],
                                 rhs=rstd[:],
                                 start=True, stop=True, skip_group_check=True)
                nmb = bcp.tile([P, P], bf16, name="nmbb", tag="bc", bufs=4)
                nc.vector.tensor_copy(nmb[:], bcps[:, 0, :])
                rsb = bcp.tile([P, P], bf16, name="rsbb", tag="bc", bufs=4)
                nc.vector.tensor_copy(rsb[:], bcps[:, 1, :])
                for dd in range(DCH):
                    eng = nc.vector if dd % 2 == 0 else nc.gpsimd
                    osl = tout[dd][:, tc0:tc1]
                    eng.tensor_add(osl, tin[dd][:, tc0:tc1], nmb[:])
                    eng.tensor_mul(osl, osl, rsb[:])
                    eng.tensor_scalar(
                        osl, osl, g_ap[:, gcol + dd:gcol + dd + 1],
                        be_ap[:, becol + dd:becol + dd + 1],
                        op0=OP.mult, op1=OP.add)

            def post_stage(l, tch, hpTs, x, t1, xn1, t2, xnext, wo_ts,
                           w1_cache, nxt):
                """wo+res+ln1+FFN+res+ln2 for one token chunk, then either
                qkv for layer l+1 (+ kv bounce/AG) or final LN + hT bounce."""
                ballt = get_ball(l)
                tc0, tc1 = tch * P, (tch + 1) * P
                # ---- Wo + residual ----
                gaA = psum.tile([P, 4, P], f32, name="woA", tag="bank",
                                bufs=3)
                gaB = psum.tile([P, 4, P], f32, name="woB", tag="bank",
                                bufs=3)
                ops_ = [gaA[:, oc, :] if oc < 4 else gaB[:, oc - 4, :]
                        for oc in range(DCH)]
                for oc in range(DCH):
                    for hp in range(DCH):
                        for sub in range(2):
                            oh = hpTs[hp * 3 + sub]
                            nc.tensor.matmul(
                                ops_[oc],
                                lhsT=wo_ts[hp][:, sub, oc * P:(oc + 1) * P],
                                rhs=oh[:],
                                start=(hp == 0 and sub == 0),
                                stop=(hp == DCH - 1 and sub == 1),
                                skip_group_check=True)
                for oc in range(DCH):
                    nc.vector.scalar_tensor_tensor(
                        t1[oc][:, tc0:tc1], ops_[oc],
                        ballt[:, BO + oc:BO + oc + 1], x[oc][:, tc0:tc1],
                        op0=OP.add, op1=OP.add)
                ln_stage(tch, t1, ballt, G1, ballt, BE1, xn1)
                # ---- FFN ----
                ypA = psum.tile([P, 4, P], f32, name="ypA", tag="yps",
                                bufs=2)
                ypB = psum.tile([P, 4, P], f32, name="ypB", tag="yps",
                                bufs=2)
                yps = [ypA[:, oc, :] if oc < 4 else ypB[:, oc - 4, :]
                       for oc in range(DCH)]
                for yp in (ypA, ypB):
                    nc.tensor.matmul(
                        yp[:].rearrange("p a b -> p (a b)"),
                        lhsT=ones_row_b[:], rhs=zrow[:],
                        start=True, stop=True, skip_group_check=True)
                fpb = None
                prevf = None
                for og in range(4):
                    w1t = w1_cache[og]
                    for j in range(DCH):
                        fc = og * DCH + j
                        if fc % 2 == 0:
                            w2t = w2p.tile([P, 2, D], bf16, name="w2t",
                                           tag="w2", bufs=6)
                            nc.gpsimd.dma_start(w2t[:], w2_r[l, fc:fc + 2]
                                                .rearrange("f p d -> p f d"))
                        if fc % 4 == 0:
                            fpb = psum.tile([P, 4, P], f32, name="fpb",
                                            tag="bank", bufs=3)
                        fps = fpb[:, fc % 4, :]
                        for ic in range(DCH):
                            nc.tensor.matmul(
                                fps,
                                lhsT=w1t[:, ic, j * P:(j + 1) * P],
                                rhs=xn1[ic][:, tc0:tc1],
                                start=(ic == 0), stop=(ic == DCH - 1),
                                skip_group_check=True)
                        ft = ftp.tile([P, P], bf16, name="ft", tag="ft",
                                      bufs=6)
                        if fc % 2 == 0:
                            nc.vector.tensor_scalar(
                                ft[:], fps, ballt[:, B1 + fc:B1 + fc + 1],
                                0.0, op0=OP.add, op1=OP.max)
                        else:
                            nc.scalar.activation(
                                ft[:], fps, AF.Relu,
                                bias=ballt[:, B1 + fc:B1 + fc + 1])
                        if prevf is not None:
                            pfc, pft, pw2t = prevf
                            for oc in range(DCH):
                                nc.tensor.matmul(
                                    yps[oc],
                                    lhsT=pw2t[:, pfc % 2,
                                              oc * P:(oc + 1) * P],
                                    rhs=pft[:],
                                    start=False, stop=False,
                                    skip_group_check=True)
                        prevf = (fc, ft, w2t)
                pfc, pft, pw2t = prevf
                for oc in range(DCH):
                    nc.tensor.matmul(
                        yps[oc],
                        lhsT=pw2t[:, pfc % 2, oc * P:(oc + 1) * P],
                        rhs=pft[:],
                        start=False, stop=True,
                        skip_group_check=True)
                for oc in range(DCH):
                    nc.vector.scalar_tensor_tensor(
                        t2[oc][:, tc0:tc1], yps[oc],
                        ballt[:, B2 + oc:B2 + oc + 1], xn1[oc][:, tc0:tc1],
                        op0=OP.add, op1=OP.add)
                ln_stage(tch, t2, ballt, G2, ballt, BE2, xnext)
                # ---- next-layer qkv or final LN ----
                if l < L - 1:
                    qn, kn, vtn, wn = nxt
                    qkv_stage(l + 1, tch, xnext, wn, qn, kn, vtn)
                else:
                    hT = nxt
                    ln_stage(tch, xnext, gft, 0, bft, 0, hT)
                    hinr = hinF[tch].rearrange("(p c) -> p c", c=KC) \
                        .rearrange("p (d t) -> p d t", d=DCH)
                    for dd in range(DCH):
                        nc.sync.dma_start(hinr[:, dd, :],
                                          hT[dd][:, tc0:tc1])
                    nc.gpsimd.collective_compute(
                        "AllGather", OP.bypass, replica_groups=ALL_GROUP,
                        ins=[hinF[tch].opt()], outs=[houtF[tch].opt()])

            def xtiles(nm):
                return [acts.tile([P, TOK], bf16, name=f"{nm}{d}", tag="x",
                                  bufs=34) for d in range(DCH)]

            def qkvtiles(l):
                return [qkvp.tile([P, DCH, TOK], bf16, name=f"{nm}{l}",
                                  tag="qkv", bufs=6)
                        for nm in ("qT", "kT", "vT")]

            # ================= embedding + layer-0 qkv =================
            x = xtiles("x0_")
            w0 = load_qkv_w(0)
            qT, kT, vT = qkvtiles(0)
            for tch in range(2):
                g = sqp.tile([P, D], bf16, name="embrow", tag="emb", bufs=2)
                nc.gpsimd.indirect_dma_start(
                    out=g[:], out_offset=None, in_=embs[:],
                    in_offset=bass.IndirectOffsetOnAxis(
                        ap=tokt[:, tch:tch + 1], axis=0))
                tpb = None
                for dd in range(DCH):
                    if dd % 4 == 0:
                        tpb = psum.tile([P, 4, P], bf16, name="etp",
                                        tag="misc", bufs=1)
                    tp = tpb[:, dd % 4, :]
                    nc.tensor.transpose(tp, g[:, dd * P:(dd + 1) * P],
                                        ident_b[:])
                    nc.vector.tensor_add(
                        x[dd][:, tch * P:(tch + 1) * P], tp,
                        peTt[:, dd, tch * P:(tch + 1) * P])
                qkv_stage(0, tch, x, w0, qT, kT, vT)

            # ================= transformer layers =================
            for l in range(L):
                wo_ts = []
                for hp in range(DCH):
                    wt = wop.tile([DK, 2, D], bf16, name=f"wo{l}_{hp}",
                                  tag="wo", bufs=7)
                    nc.scalar.dma_start(wt[:], wo_r[l, hp])
                    wo_ts.append(wt)
                w1_cache = {}
                for og in range(4):
                    wt = w1p.tile([P, DCH, DCH * P], bf16,
                                  name=f"w1_{l}_{og}", tag="w1", bufs=4)
                    nc.scalar.dma_start(wt[:], w1_r[l, og])
                    w1_cache[og] = wt
                if l < L - 1:
                    wn = load_qkv_w(l + 1)
                    qn, kn, vtn = qkvtiles(l + 1)
                    nxt = (qn, kn, vtn, wn)
                else:
                    nxt = xtiles("hT_")
                t1, xn1, t2, xnext = (xtiles(f"t1_{l}"), xtiles(f"xn1_{l}"),
                                      xtiles(f"t2_{l}"), xtiles(f"x{l + 1}_"))

                KT0, V0 = load_kv(l, 0)
                KT1, V1 = load_kv(l, 1)
                with nc.named_scope(f"L{l}attA"):
                    hpA = attn_stage(0, qT, KT0, V0, None, None)
                with nc.named_scope(f"L{l}postA"):
                    post_stage(l, 0, hpA, x, t1, xn1, t2, xnext, wo_ts,
                               w1_cache, nxt)
                with nc.named_scope(f"L{l}attB"):
                    hpB = attn_stage(1, qT, KT0, V0, KT1, V1)
                with nc.named_scope(f"L{l}postB"):
                    post_stage(l, 1, hpB, x, t1, xn1, t2, xnext, wo_ts,
                               w1_cache, nxt)
                x = xnext
                if l < L - 1:
                    qT, kT, vT = nxt[0], nxt[1], nxt[2]

        # ================= vocab-parallel head =================
        with ExitStack() as hctx:
            htp = hctx.enter_context(tc.tile_pool(name="htp", bufs=12))
            wvp = hctx.enter_context(tc.tile_pool(name="wvp", bufs=7))
            otp = hctx.enter_context(tc.tile_pool(name="otp", bufs=3))

            HT = [[None] * DCH for _ in range(2)]
            for half in range(2):
                houtr = houtF[half].rearrange("(j p c) -> j p c", p=P, c=KC)
                for dd in range(DCH):
                    ht = htp.tile([P, NCORE * P], bf16,
                                  name=f"HT{half}_{dd}", tag="ht", bufs=12)
                    nc.sync.dma_start(
                        ht[:].rearrange("p (j t) -> p j t", j=NCORE),
                        houtr[:, :, dd * P:(dd + 1) * P]
                        .rearrange("j p t -> p j t"))
                    HT[half][dd] = ht

            for vh in range(2):
                wts = []
                for dd in range(DCH):
                    wt = wvp.tile([P, VPAD // 2], bf16, name="woutt",
                                  tag="wv", bufs=7)
                    nc.scalar.dma_start(
                        wt[:], woutc[dd * P:(dd + 1) * P,
                                     vh * (VPAD // 2):(vh + 1) * (VPAD // 2)])
                    wts.append(wt)
                for vc in range(VCH // 2):
                    vch = vh * (VCH // 2) + vc
                    ot = otp.tile([P, 2048], f32, name="lsb", tag="lsb",
                                  bufs=3)
                    qi = 0
                    pend = None
                    for half in range(2):
                        for tq in range(2):
                            lp = psum.tile([P, 512], f32, name="logps",
                                           tag="bank", bufs=3)
                            for dd in range(DCH):
                                nc.tensor.matmul(
                                    lp[:],
                                    lhsT=wts[dd][:, vc * P:(vc + 1) * P],
                                    rhs=HT[half][dd][:, tq * 512:
                                                     (tq + 1) * 512],
                                    start=(dd == 0), stop=(dd == DCH - 1),
                                    skip_group_check=True)
                            if pend is not None:
                                plp, posl, pqi = pend
                                if pqi % 2 == 0:
                                    nc.scalar.activation(
                                        posl, plp[:], AF.Identity,
                                        bias=boutt[:, vch:vch + 1])
                                else:
                                    nc.vector.tensor_scalar_add(
                                        posl, plp[:], boutt[:, vch:vch + 1])
                            osl = ot[:, half * 1024 + tq * 512:
                                     half * 1024 + (tq + 1) * 512]
                            pend = (lp, osl, qi + vc)
                            qi += 1
                    plp, posl, pqi = pend
                    if pqi % 2 == 0:
                        nc.scalar.activation(
                            posl, plp[:], AF.Identity,
                            bias=boutt[:, vch:vch + 1])
                    else:
                        nc.vector.tensor_scalar_add(
                            posl, plp[:], boutt[:, vch:vch + 1])
                    eng = nc.sync if vch % 2 == 0 else nc.gpsimd
                    eng.dma_start(out[vch * P:(vch + 1) * P, :], ot[:])

    return nc


_CACHED = {}


def _compiled():
    if "nc" not in _CACHED:
        nc = bacc.Bacc("TRN2", target_bir_lowering=False, debug=False,
                       num_devices=NCORE)
        build(nc)
        nc.compile()
        _CACHED["nc"] = nc
    return _CACHED["nc"]


def _bf(a):
    return np.ascontiguousarray(np.asarray(a, np.float32)).astype(
        ml_dtypes.bfloat16)


def _make_inputs(tokens, emb, pe, wq, bq, wk, bk, wv, bv, wo, bo,
                 w1, b1, w2, b2, g1, be1, g2, be2, gf, bf, wout, bout):
    f = np.float32
    tokens = np.asarray(tokens).astype(np.int32)

    def parr(b):  # [L, dim] -> [L, P, dim//P]
        b = np.asarray(b, f)
        return b.reshape(L, b.shape[1] // P, P).transpose(0, 2, 1)

    def parr1(b):  # [dim] -> [P, dim//P]
        b = np.asarray(b, f)
        return np.ascontiguousarray(b.reshape(b.shape[0] // P, P).T)

    ball = np.concatenate(
        [parr(bq), parr(bk), parr(bv), parr(bo), parr(b2),
         parr(g1), parr(be1), parr(g2), parr(be2), parr(b1)], axis=2)

    # wqkv [L, DCH, P, 3, D]: lhsT chunks, natural rows
    wq_, wk_, wv_ = (np.asarray(w, f).reshape(L, DCH, P, D)
                     for w in (wq, wk, wv))
    wqkv = _bf(np.stack([wq_, wk_, wv_], axis=3))
    wo_r = _bf(np.asarray(wo, f).reshape(L, DCH, 2, DK, D)
               .transpose(0, 1, 3, 2, 4))
    # w1 [L, D, F] -> [L, 4, P(ic-row), DCH(ic), 768]
    w1_ = np.asarray(w1, f).reshape(L, DCH, P, 4, DCH * P)
    w1_r = _bf(w1_.transpose(0, 3, 2, 1, 4))
    w2_r = _bf(np.asarray(w2, f).reshape(L, FCH, P, D))

    emb_s = _bf(np.asarray(emb, f) * SQD)
    pe = np.asarray(pe, f)
    wout = np.asarray(wout, f)
    bout = np.asarray(bout, f)

    common = {
        "embs": emb_s, "wqkv": wqkv, "wo_r": wo_r, "w1_r": w1_r,
        "w2_r": w2_r, "ball": np.ascontiguousarray(ball),
        "gfp": parr1(gf), "bfp": parr1(bf),
    }

    tri = np.where(np.arange(P)[:, None] <= np.arange(P)[None, :],
                   0.0, -1e9).astype(f)  # [k, q]: visible iff k <= q
    zeros = np.zeros((P, P), f)
    neg = np.full((P, P), -1e9, f)

    in_maps = []
    for c in range(NCORE):
        b, r = divmod(c, GRP)
        chunks = (r, 7 - r)
        rows = np.concatenate(
            [np.arange(ch * P, (ch + 1) * P) for ch in chunks])
        tok_c = np.stack(
            [tokens[b, ch * P:(ch + 1) * P] for ch in chunks], axis=1
        ).astype(np.int32)
        peT_c = np.ascontiguousarray(pe[rows].T)  # [D, TOK]
        peTr = _bf(peT_c.reshape(DCH, P, TOK).transpose(1, 0, 2))

        # additive masks: amask[0][s] for qc1=chunk r vs key chunk s;
        # amask[1][g] for qc2=chunk 7-r vs key chunk 7-g.
        am = np.empty((2, 4, P, P), f)
        for s in range(4):
            am[0, s] = zeros if s < r else (tri if s == r else neg)
        for g2_ in range(4):
            am[1, g2_] = zeros if g2_ > r else (tri if g2_ == r else neg)

        wslice = np.zeros((D, VPAD), f)
        wslice[:, :VSH] = wout[:, c * VSH:(c + 1) * VSH]
        bslice = np.zeros((VPAD,), f)
        bslice[:VSH] = bout[c * VSH:(c + 1) * VSH]
        boutp_c = np.ascontiguousarray(bslice.reshape(VCH, P).T)

        m = dict(common)
        m.update({
            "tok": tok_c,
            "peTr": peTr,
            "amask": _bf(am),
            "woutc": _bf(wslice),
            "boutp": boutp_c,
        })
        in_maps.append(m)
    return in_maps


def run(in_maps, **kwargs):
    nc = _compiled()
    return run_bass_kernel_spmd(nc, in_maps, list(range(NCORE)), **kwargs)


def assemble(results):
    """results[c]['out'] [VPAD, 8*TOK] -> full logits [B, S, V].

    out col = half*1024 + j*128 + t, where half selects the token chunk
    (rank j owns chunks (j%4, 7-j%4)) and j is the source core.
    """
    full = np.empty((B, S, V), np.float32)
    for c in range(NCORE):
        lt = np.asarray(results[c]["out"])[:VSH]  # [4000, 2048]
        lg = lt.T  # [2048, 4000]
        for j in range(NCORE):
            bj, rj = divmod(j, GRP)
            for half, ch in enumerate((rj, 7 - rj)):
                full[bj, ch * P:(ch + 1) * P, c * VSH:(c + 1) * VSH] = \
                    lg[half * 1024 + j * P:half * 1024 + (j + 1) * P]
    return full


def kernel(**inputs):
    in_maps = _make_inputs(**inputs)
    res = run(in_maps)
    return assemble(res.results)


# revision 28
# speedup vs baseline: 1.2131x; 1.0646x over previous
"""MinimalGPT forward on 8 Trainium2 NeuronCores — v2.

Sharding: sequence-parallel transformer + vocab-parallel head (zigzag).
  core c: batch b=c//4, rank r=c%4, owns seq chunks (r, 7-r) = 2x128 tokens.

v2 changes vs v1:
  - bf16 weights + activations end-to-end (fp32 PSUM accumulate, fp32 LN
    stats); halves HBM + collective bytes, enables 128-wide matmuls at
    full PE rate.
  - causal structure: qc1 (chunk r) attends only to chunks 0-3, qc2 only
    to 0..7-r; additive masks applied as identity-matmuls into the score
    PSUM (uniform SPMD program, per-core mask data).
  - softmax denominators ride the AV matmul (ones column packed into V);
    reciprocal via DVE reciprocal_approx_fast on head-pairs.
  - per-layer kv AllGather split into two (one per token chunk), issued
    as soon as that chunk's kv is projected; attention over chunks 0-3
    starts after AG0, hiding most collective latency behind compute.
  - ACT engine uses a single fn table (exp/ln/square/identity); LN rstd
    computed as exp(-0.5*ln(var+eps)).
"""

import math
import os
import numpy as np
import ml_dtypes
from contextlib import ExitStack

import concourse.bass as bass
import concourse.tile as tile
from concourse import bacc, mybir
from concourse.bass_utils import run_bass_kernel_spmd
from concourse.masks import make_identity

f32 = mybir.dt.float32
bf16 = mybir.dt.bfloat16
i32 = mybir.dt.int32
AF = mybir.ActivationFunctionType
OP = mybir.AluOpType

V, D, H, L, F = 32000, 768, 12, 6, 3072
B, S = 2, 1024
P = 128
DK = 64
DCH = D // P           # 6
FCH = F // P           # 24
TOK = 256              # tokens per core (2 chunks of 128)
NCORE, GRP = 8, 4
VPAD = 4096
VCH = VPAD // P        # 32
VSH = V // NCORE       # 4000
EPS = 1e-5
SQD = math.sqrt(D)
ISQDK = 1.0 / math.sqrt(DK)
HV = H * (DK + 1)      # 780: natural V cols incl per-head ones column
KC = DCH * P           # 768
KVC = KC + HV          # 1548 bounce cols per token chunk

# packed per-layer bias/gain columns in `ball` [L, P, 78]
BQ, BK, BV, BO, B2, G1, BE1, G2, BE2, B1 = 0, 6, 12, 18, 24, 30, 36, 42, 48, 54

KV_GROUPS = [[0, 1, 2, 3], [4, 5, 6, 7]]
ALL_GROUP = [list(range(NCORE))]


def build(nc):
    def din(name, shape, dt=f32):
        return nc.dram_tensor(name, shape, dt, kind="ExternalInput").ap()

    tok = din("tok", [P, 2], i32)
    peTr = din("peTr", [P, DCH, TOK], bf16)
    embs = din("embs", [V, D], bf16)           # pre-scaled by sqrt(D)
    amask = din("amask", [2, 4, P, P], bf16)   # additive score masks
    wqkv = din("wqkv", [L, DCH, P, 3, D], bf16)
    wo_r = din("wo_r", [L, DCH, DK, 2, D], bf16)
    w1_r = din("w1_r", [L, 4, P, DCH, DCH * P], bf16)
    w2_r = din("w2_r", [L, FCH, P, D], bf16)
    ball = din("ball", [L, P, 78])
    gfp = din("gfp", [P, DCH])
    bfp = din("bfp", [P, DCH])
    woutc = din("woutc", [D, VPAD], bf16)
    boutp = din("boutp", [P, VCH])

    out = nc.dram_tensor("out", [VPAD, NCORE * TOK], f32,
                         kind="ExternalOutput").ap()

    kvins = [[nc.dram_tensor(f"kvi{l}_{t}", [P * KVC], bf16).ap()
              for t in range(2)] for l in range(L)]
    kvouts = [[nc.dram_tensor(f"kvout{l}_{t}", [GRP * P * KVC], bf16).ap()
               for t in range(2)] for l in range(L)]
    hinF = [nc.dram_tensor(f"hinF{t}", [P * KC], bf16).ap() for t in range(2)]
    dwi = [nc.dram_tensor(f"dwi{i}", [256], bf16).ap() for i in range(2)]
    dwo = [nc.dram_tensor("dwo0", [256 * GRP], bf16).ap(),
           nc.dram_tensor("dwo1", [256 * NCORE], bf16,
                          addr_space="Shared").ap()]
    houtF = [nc.dram_tensor(f"houtF{t}", [NCORE * P * KC], bf16,
                            addr_space="Shared").ap() for t in range(2)]

    with tile.TileContext(
            nc, trace_sim=os.environ.get("TRACE_SIM", "0") == "1",
    ) as tc, ExitStack() as octx, \
            nc.allow_low_precision(reason="bf16 datapath, fp32 accumulate"):
        const = octx.enter_context(tc.tile_pool(name="const", bufs=1))
        stats = octx.enter_context(tc.tile_pool(name="stats", bufs=10))
        # PSUM: 8 bank-slots total (every slot pads to a full 2KB bank):
        # bank(2) scores/logits, oT(1), ga(2) qkv/wo outs, yps(2), misc(1)
        psum = octx.enter_context(
            tc.tile_pool(name="psum", bufs=1, space="PSUM"))

        def ctile(shape, dt, nm):
            return const.tile(shape, dt, name=nm, tag=nm)

        ident_f = ctile([P, P], f32, "ident_f")
        make_identity(nc, ident_f[:])
        ident_b = ctile([P, P], bf16, "ident_b")
        nc.vector.tensor_copy(ident_b[:], ident_f[:])
        ones_col_b = ctile([P, 1], bf16, "ones_col_b")
        nc.vector.memset(ones_col_b[:], 1.0)
        ones_row_b = ctile([1, P], bf16, "ones_row_b")
        nc.vector.memset(ones_row_b[:], 1.0)
        zrow = ctile([1, 4 * P], bf16, "zrow")
        nc.vector.memset(zrow[:], 0.0)
        one_i = ctile([1, 1], i32, "one_i")
        nc.vector.memset(one_i[:], 1)
        magic_row = ctile([1, P], i32, "magic_row")
        nc.vector.memset(magic_row[:], 0x5F3759DF)
        eps_t = ctile([1, 1], f32, "eps_t")
        nc.vector.memset(eps_t[:], EPS)
        # tiny dummy collectives to warm the CC stack while embedding runs
        nc.gpsimd.collective_compute(
            "AllGather", OP.bypass, replica_groups=KV_GROUPS,
            ins=[dwi[0].opt()], outs=[dwo[0].opt()])
        nc.gpsimd.collective_compute(
            "AllGather", OP.bypass, replica_groups=ALL_GROUP,
            ins=[dwi[1].opt()], outs=[dwo[1].opt()])
        tokt = ctile([P, 2], i32, "tokt")
        nc.sync.dma_start(tokt[:], tok[:])
        mt = []
        for qi in range(2):
            row = []
            for s in range(4):
                m = ctile([P, P], bf16, f"mask{qi}_{s}")
                nc.sync.dma_start(m[:], amask[qi, s])
                row.append(m)
            mt.append(row)
        peTt = ctile([P, DCH, TOK], bf16, "peTt")
        nc.sync.dma_start(peTt[:], peTr[:])
        gft = ctile([P, DCH], f32, "gft")
        nc.sync.dma_start(gft[:], gfp[:])
        bft = ctile([P, DCH], f32, "bft")
        nc.sync.dma_start(bft[:], bfp[:])
        boutt = ctile([P, VCH], f32, "boutt")
        nc.sync.dma_start(boutt[:], boutp[:])

        with ExitStack() as lctx:
            acts = lctx.enter_context(tc.tile_pool(name="acts", bufs=34))
            sqp = lctx.enter_context(tc.tile_pool(name="sqp", bufs=4))
            bcp = lctx.enter_context(tc.tile_pool(name="bcp", bufs=4))
            qkvp = lctx.enter_context(tc.tile_pool(name="qkvp", bufs=6))
            vna = lctx.enter_context(tc.tile_pool(name="vna", bufs=3))
            ktp = lctx.enter_context(tc.tile_pool(name="ktp", bufs=9))
            vp = lctx.enter_context(tc.tile_pool(name="vp", bufs=9))
            ep = lctx.enter_context(tc.tile_pool(name="ep", bufs=3))
            hpp = lctx.enter_context(tc.tile_pool(name="hpp", bufs=8))
            ftp = lctx.enter_context(tc.tile_pool(name="ftp", bufs=6))
            wqp = lctx.enter_context(tc.tile_pool(name="wqp", bufs=7))
            wop = lctx.enter_context(tc.tile_pool(name="wop", bufs=7))
            w1p = lctx.enter_context(tc.tile_pool(name="w1p", bufs=4))
            w2p = lctx.enter_context(tc.tile_pool(name="w2p", bufs=6))
            bpool = lctx.enter_context(tc.tile_pool(name="bpool", bufs=3))

            ballts = {}

            def get_ball(l):
                if l not in ballts:
                    t = bpool.tile([P, 78], f32, name=f"ball{l}", tag="ball",
                                   bufs=3)
                    nc.sync.dma_start(t[:], ball[l])
                    ballts[l] = t
                return ballts[l]

            def load_qkv_w(l):
                wts = []
                for ic in range(DCH):
                    wt = wqp.tile([P, 3, D], bf16, name=f"wqkv{l}_{ic}",
                                  tag="wqkv", bufs=7)
                    nc.scalar.dma_start(wt[:], wqkv[l, ic])
                    wts.append(wt)
                return wts

            def _proj(pi, bcol, dst, xin, wts, ballt, tc0, tc1):
                gaA = psum.tile([P, 4, P], f32, name="gaA", tag="bank",
                                bufs=3)
                gaB = psum.tile([P, 4, P], f32, name="gaB", tag="bank",
                                bufs=3)
                outs = [gaA[:, oc, :] if oc < 4 else gaB[:, oc - 4, :]
                        for oc in range(DCH)]
                for oc in range(DCH):
                    for ic in range(DCH):
                        nc.tensor.matmul(
                            outs[oc],
                            lhsT=wts[ic][:, pi, oc * P:(oc + 1) * P],
                            rhs=xin[ic][:, tc0:tc1],
                            start=(ic == 0), stop=(ic == DCH - 1),
                            skip_group_check=True,
                        )
                for oc in range(DCH):
                    nc.vector.tensor_scalar_add(
                        dst[:, oc, tc0:tc1], outs[oc],
                        ballt[:, bcol + oc:bcol + oc + 1])

            def qkv_stage(l, tch, xin, wts, qTt, kTt, vTt):
                """Project x(tch) -> k,v first, bounce+AllGather, then q."""
                ballt = get_ball(l)
                tc0, tc1 = tch * P, (tch + 1) * P
                _proj(1, BK, kTt, xin, wts, ballt, tc0, tc1)
                _proj(2, BV, vTt, xin, wts, ballt, tc0, tc1)

                # natural V (+ ones cols) for this token chunk
                vn = vna.tile([P, HV], bf16, name=f"vn{l}_{tch}", tag="vn",
                              bufs=3)
                nc.gpsimd.memset(
                    vn[:].rearrange("p (h c) -> p h c", h=H)[:, :, DK:], 1.0)
                tpb = None
                for dd in range(DCH):
                    if dd % 4 == 0:
                        tpb = psum.tile([P, 4, P], bf16, name="vtp",
                                        tag="misc", bufs=1)
                    tp = tpb[:, dd % 4, :]
                    nc.tensor.transpose(tp, vTt[:, dd, tc0:tc1], ident_b[:])
                    for j in range(2):
                        h = 2 * dd + j
                        nc.scalar.activation(
                            vn[:, h * (DK + 1):h * (DK + 1) + DK],
                            tp[:, j * DK:(j + 1) * DK], AF.Identity)
                bounce_kv(l, tch, kTt, vn)
                _proj(0, BQ, qTt, xin, wts, ballt, tc0, tc1)

            def bounce_kv(l, tch, kTt, vn):
                kvi = kvins[l][tch].rearrange("(p c) -> p c", c=KVC)
                nc.sync.dma_start(
                    kvi[:, 0:KC].rearrange("p (d t) -> p d t", d=DCH),
                    kTt[:, :, tch * P:(tch + 1) * P])
                nc.sync.dma_start(kvi[:, KC:], vn[:])
                nc.gpsimd.collective_compute(
                    "AllGather", OP.bypass, replica_groups=KV_GROUPS,
                    ins=[kvins[l][tch].opt()], outs=[kvouts[l][tch].opt()])

            def load_kv(l, half):
                kvo = kvouts[l][half].rearrange("(g p c) -> g p c", p=P, c=KVC)
                KTs, Vs = [], []
                for g in range(GRP):
                    kt = ktp.tile([P, DCH, P], bf16, name=f"KT{half}_{g}",
                                  tag="kt", bufs=9)
                    nc.sync.dma_start(
                        kt[:], kvo[g, :, 0:KC].rearrange(
                            "p (d t) -> p d t", d=DCH))
                    KTs.append(kt)
                    v = vp.tile([P, HV], bf16, name=f"V{half}_{g}", tag="v",
                                bufs=9)
                    nc.sync.dma_start(v[:], kvo[g, :, KC:])
                    Vs.append(v)
                return KTs, Vs

            def attn_stage(qc, qTt, KT0, V0, KT1, V1):
                """Attention for query chunk qc, software-pipelined: the
                score matmuls of unit i+1 are emitted before the AV matmuls
                of unit i so the PE never waits on the exp (ACT) feedback."""
                qc0, qc1_ = qc * P, (qc + 1) * P
                hpTs = []
                aoTs = {}

                def build(hp, sub):
                    h = 2 * hp + sub
                    hr = sub * DK
                    banks = []
                    bank0 = psum.tile([P, 4 * P], f32, name="sc0",
                                      tag="bank", bufs=3)
                    for s in range(GRP):
                        nc.tensor.matmul(
                            bank0[:, s * P:(s + 1) * P],
                            lhsT=KT0[s][hr:hr + DK, hp, :],
                            rhs=qTt[hr:hr + DK, hp, qc0:qc1_],
                            start=True, stop=True,
                            skip_group_check=True)
                    banks.append((bank0, V0, mt[0] if qc == 0 else None))
                    if qc == 1:
                        bank1 = psum.tile([P, 4 * P], f32, name="sc1",
                                          tag="bank", bufs=3)
                        for s in range(GRP):
                            nc.tensor.matmul(
                                bank1[:, s * P:(s + 1) * P],
                                lhsT=KT1[s][hr:hr + DK, hp, :],
                                rhs=qTt[hr:hr + DK, hp, qc0:qc1_],
                                start=True, stop=True,
                                skip_group_check=True)
                        banks.append((bank1, V1, mt[1]))
                    es = []
                    for bank, _vs, msk in banks:
                        e = ep.tile([P, 4 * P], bf16, name="e", tag="e",
                                    bufs=3)
                        nc.scalar.activation(e[:], bank[:], AF.Exp,
                                             scale=ISQDK)
                        if msk is not None:
                            for s in range(GRP):
                                nc.gpsimd.tensor_mul(
                                    e[:, s * P:(s + 1) * P],
                                    e[:, s * P:(s + 1) * P], msk[s][:])
                        es.append(e)
                    return (hp, sub, banks, es)

                def do_av(item):
                    hp, sub, banks, es = item
                    h = 2 * hp + sub
                    if sub == 0:
                        aoTs[hp] = psum.tile([P, 4, P], f32, name="aoT",
                                             tag="aoT", bufs=2)
                    oT = aoTs[hp][0:DK + 1, sub, :]
                    nbl = len(banks) * GRP
                    bi = 0
                    for (bank, vs, _m), e in zip(banks, es):
                        for s in range(GRP):
                            nc.tensor.matmul(
                                oT,
                                lhsT=vs[s][:, h * (DK + 1):
                                           (h + 1) * (DK + 1)],
                                rhs=e[:, s * P:(s + 1) * P],
                                start=(bi == 0), stop=(bi == nbl - 1),
                                skip_group_check=True)
                            bi += 1

                def finish(hp):
                    aoT = aoTs.pop(hp)
                    oT2 = aoT[0:DK + 1, 0:2, :]
                    den = stats.tile([1, 2, P], f32, name="den", tag="st")
                    nc.vector.tensor_copy(den[:, 0, :],
                                          oT2[DK:DK + 1, 0, :])
                    nc.vector.tensor_copy(den[:, 1, :],
                                          oT2[DK:DK + 1, 1, :])
                    rec = stats.tile([1, 2 * P], f32, name="rec", tag="st")
                    nc.vector.reciprocal_approx_fast(
                        rec[:], den[:].rearrange("a s p -> a (s p)"))
                    recb = stats.tile([1, 2 * P], bf16, name="recb",
                                      tag="st")
                    nc.vector.tensor_copy(recb[:], rec[:])
                    rb = aoT[0:DK, 2:4, :]
                    nc.tensor.matmul(rb, lhsT=ones_row_b[:, 0:DK],
                                     rhs=recb[:], start=True, stop=True,
                                     skip_group_check=True)
                    rbs = bcp.tile([DK, 2 * P], bf16, name="rbs", tag="bc",
                                   bufs=4)
                    nc.vector.tensor_copy(
                        rbs[:], rb.rearrange("p a b -> p (a b)"))
                    for sub in range(2):
                        oh = hpp.tile([DK, P], bf16, name=f"oh{hp}_{sub}",
                                      tag="oh", bufs=14)
                        nc.vector.tensor_mul(oh[:], oT2[0:DK, sub, :],
                                             rbs[:, sub * P:(sub + 1) * P])
                        hpTs.append(oh)
                    hpTs.append(None)

                prev = None
                for hp in range(DCH):
                    for sub in range(2):
                        item = build(hp, sub)
                        if prev is not None:
                            do_av(prev)
                            if prev[1] == 1:
                                finish(prev[0])
                        prev = item
                do_av(prev)
                finish(prev[0])
                return hpTs

            def ln_stage(tch, tin, g_ap, gcol, be_ap, becol, tout):
                """LayerNorm over features for one token chunk.

                tin: 6 [P, TOK] bf16 tiles (reads [:, tch*P:+P]);
                tout: 6 tiles (writes same slice).
                """
                tc0, tc1 = tch * P, (tch + 1) * P
                stp = psum.tile([1, 2, P], f32, name="stp", tag="misc",
                                bufs=1)
                st_s = stp[:, 0, :]
                st_q = stp[:, 1, :]
                sqs = []
                for dd in range(DCH):
                    sq = sqp.tile([P, P], bf16, name="sq", tag="sq", bufs=6)
                    nc.scalar.activation(sq[:], tin[dd][:, tc0:tc1], AF.Square)
                    sqs.append(sq)
                for dd in range(DCH):
                    nc.tensor.matmul(st_s, lhsT=ones_col_b[:],
                                     rhs=tin[dd][:, tc0:tc1],
                                     start=(dd == 0), stop=(dd == DCH - 1),
                                     skip_group_check=True)
                for dd in range(DCH):
                    nc.tensor.matmul(st_q, lhsT=ones_col_b[:], rhs=sqs[dd][:],
                                     start=(dd == 0), stop=(dd == DCH - 1),
                                     skip_group_check=True)
                nm = stats.tile([1, P], bf16, name="nm", tag="st")
                nc.vector.tensor_scalar_mul(nm[:], st_s, -1.0 / D)
                m2 = stats.tile([1, P], f32, name="m2", tag="st")
                nc.vector.tensor_mul(m2[:], nm[:], nm[:])
                ex2 = stats.tile([1, P], f32, name="ex2", tag="st")
                nc.vector.tensor_scalar_mul(ex2[:], st_q, 1.0 / D)
                ve = stats.tile([1, P], f32, name="ve", tag="st")
                nc.vector.tensor_sub(ve[:], ex2[:], m2[:])
                nc.vector.tensor_scalar_add(ve[:], ve[:], EPS)
                vh = stats.tile([1, P], f32, name="vh", tag="st")
                nc.vector.tensor_scalar_mul(vh[:], ve[:], 0.5)
                yi = stats.tile([1, P], i32, name="yi", tag="st")
                nc.vector.tensor_scalar(yi[:], ve[:].bitcast(i32), one_i[:],
                                        None, op0=OP.arith_shift_right)
                nc.vector.tensor_sub(yi[:], magic_row[:], yi[:])
                y = yi[:].bitcast(f32)
                t = stats.tile([1, P], f32, name="t", tag="st")
                a = stats.tile([1, P], f32, name="a", tag="st")
                for _ in range(2):
                    nc.vector.tensor_mul(t[:], y, y)
                    nc.vector.tensor_mul(t[:], t[:], vh[:])
                    nc.vector.tensor_scalar(a[:], t[:], -1.0, 1.5,
                                            op0=OP.mult, op1=OP.add)
                    nc.vector.tensor_mul(y, a[:], y)
                rstd = stats.tile([1, P], bf16, name="rstd", tag="st")
                nc.vector.tensor_copy(rstd[:], y)
                bcps = psum.tile([P, 2, P], f32, name="bcps", tag="misc",
                                 bufs=1)
                nc.tensor.matmul(bcps[:, 0, :], lhsT=ones_row_b[:], rhs=nm[:],
                                 start=True, stop=True, skip_group_check=True)
                nc.tensor.matmul(bcps[:, 1, :], lhsT=ones_row_b[:],
                                 rhs=rstd[:],
                                 start=True, stop=True, skip_group_check=True)
                nmb = bcp.tile([P, P], bf16, name="nmbb", tag="bc", bufs=4)
                nc.vector.tensor_copy(nmb[:], bcps[:, 0, :])
                rsb = bcp.tile([P, P], bf16, name="rsbb", tag="bc", bufs=4)
                nc.vector.tensor_copy(rsb[:], bcps[:, 1, :])
                for dd in range(DCH):
                    eng = nc.vector if dd % 2 == 0 else nc.gpsimd
                    osl = tout[dd][:, tc0:tc1]
                    eng.tensor_add(osl, tin[dd][:, tc0:tc1], nmb[:])
                    eng.tensor_mul(osl, osl, rsb[:])
                    eng.tensor_scalar(
                        osl, osl, g_ap[:, gcol + dd:gcol + dd + 1],
                        be_ap[:, becol + dd:becol + dd + 1],
                        op0=OP.mult, op1=OP.add)

            def post_stage(l, tch, hpTs, x, t1, xn1, t2, xnext, wo_ts,
                           w1_cache, nxt):
                """wo+res+ln1+FFN+res+ln2 for one token chunk, then either
                qkv for layer l+1 (+ kv bounce/AG) or final LN + hT bounce."""
                ballt = get_ball(l)
                tc0, tc1 = tch * P, (tch + 1) * P
                # ---- Wo + residual ----
                gaA = psum.tile([P, 4, P], f32, name="woA", tag="bank",
                                bufs=3)
                gaB = psum.tile([P, 4, P], f32, name="woB", tag="bank",
                                bufs=3)
                ops_ = [gaA[:, oc, :] if oc < 4 else gaB[:, oc - 4, :]
                        for oc in range(DCH)]
                for oc in range(DCH):
                    for hp in range(DCH):
                        for sub in range(2):
                            oh = hpTs[hp * 3 + sub]
                            nc.tensor.matmul(
                                ops_[oc],
                                lhsT=wo_ts[hp][:, sub, oc * P:(oc + 1) * P],
                                rhs=oh[:],
                                start=(hp == 0 and sub == 0),
                                stop=(hp == DCH - 1 and sub == 1),
                                skip_group_check=True)
                for oc in range(DCH):
                    nc.vector.scalar_tensor_tensor(
                        t1[oc][:, tc0:tc1], ops_[oc],
                        ballt[:, BO + oc:BO + oc + 1], x[oc][:, tc0:tc1],
                        op0=OP.add, op1=OP.add)
                ln_stage(tch, t1, ballt, G1, ballt, BE1, xn1)
                # ---- FFN ----
                ypA = psum.tile([P, 4, P], f32, name="ypA", tag="yps",
                                bufs=2)
                ypB = psum.tile([P, 4, P], f32, name="ypB", tag="yps",
                                bufs=2)
                yps = [ypA[:, oc, :] if oc < 4 else ypB[:, oc - 4, :]
                       for oc in range(DCH)]
                for yp in (ypA, ypB):
                    nc.tensor.matmul(
                        yp[:].rearrange("p a b -> p (a b)"),
                        lhsT=ones_row_b[:], rhs=zrow[:],
                        start=True, stop=True, skip_group_check=True)
                fpb = None
                prevf = None
                for og in range(4):
                    w1t = w1_cache[og]
                    for j in range(DCH):
                        fc = og * DCH + j
                        if fc % 2 == 0:
                            w2t = w2p.tile([P, 2, D], bf16, name="w2t",
                                           tag="w2", bufs=6)
                            nc.gpsimd.dma_start(w2t[:], w2_r[l, fc:fc + 2]
                                                .rearrange("f p d -> p f d"))
                        if fc % 4 == 0:
                            fpb = psum.tile([P, 4, P], f32, name="fpb",
                                            tag="bank", bufs=3)
                        fps = fpb[:, fc % 4, :]
                        for ic in range(DCH):
                            nc.tensor.matmul(
                                fps,
                                lhsT=w1t[:, ic, j * P:(j + 1) * P],
                                rhs=xn1[ic][:, tc0:tc1],
                                start=(ic == 0), stop=(ic == DCH - 1),
                                skip_group_check=True)
                        ft = ftp.tile([P, P], bf16, name="ft", tag="ft",
                                      bufs=6)
                        if fc % 2 == 0:
                            nc.vector.tensor_scalar(
                                ft[:], fps, ballt[:, B1 + fc:B1 + fc + 1],
                                0.0, op0=OP.add, op1=OP.max)
                        else:
                            nc.scalar.activation(
                                ft[:], fps, AF.Relu,
                                bias=ballt[:, B1 + fc:B1 + fc + 1])
                        if prevf is not None:
                            pfc, pft, pw2t = prevf
                            for oc in range(DCH):
                                nc.tensor.matmul(
                                    yps[oc],
                                    lhsT=pw2t[:, pfc % 2,
                                              oc * P:(oc + 1) * P],
                                    rhs=pft[:],
                                    start=False, stop=False,
                                    skip_group_check=True)
                        prevf = (fc, ft, w2t)
                pfc, pft, pw2t = prevf
                for oc in range(DCH):
                    nc.tensor.matmul(
                        yps[oc],
                        lhsT=pw2t[:, pfc % 2, oc * P:(oc + 1) * P],
                        rhs=pft[:],
                        start=False, stop=True,
                        skip_group_check=True)
                for oc in range(DCH):
                    nc.vector.scalar_tensor_tensor(
                        t2[oc][:, tc0:tc1], yps[oc],
                        ballt[:, B2 + oc:B2 + oc + 1], xn1[oc][:, tc0:tc1],
                        op0=OP.add, op1=OP.add)
                ln_stage(tch, t2, ballt, G2, ballt, BE2, xnext)
                # ---- next-layer qkv or final LN ----
                if l < L - 1:
                    qn, kn, vtn, wn = nxt
                    qkv_stage(l + 1, tch, xnext, wn, qn, kn, vtn)
                else:
                    hT = nxt
                    ln_stage(tch, xnext, gft, 0, bft, 0, hT)
                    hinr = hinF[tch].rearrange("(p c) -> p c", c=KC) \
                        .rearrange("p (d t) -> p d t", d=DCH)
                    for dd in range(DCH):
                        nc.sync.dma_start(hinr[:, dd, :],
                                          hT[dd][:, tc0:tc1])
                    nc.gpsimd.collective_compute(
                        "AllGather", OP.bypass, replica_groups=ALL_GROUP,
                        ins=[hinF[tch].opt()], outs=[houtF[tch].opt()])

            def xtiles(nm):
                return [acts.tile([P, TOK], bf16, name=f"{nm}{d}", tag="x",
                                  bufs=34) for d in range(DCH)]

            def qkvtiles(l):
                return [qkvp.tile([P, DCH, TOK], bf16, name=f"{nm}{l}",
                                  tag="qkv", bufs=6)
                        for nm in ("qT", "kT", "vT")]

            # ================= embedding + layer-0 qkv =================
            x = xtiles("x0_")
            w0 = load_qkv_w(0)
            qT, kT, vT = qkvtiles(0)
            for tch in range(2):
                g = sqp.tile([P, D], bf16, name="embrow", tag="emb", bufs=2)
                nc.gpsimd.indirect_dma_start(
                    out=g[:], out_offset=None, in_=embs[:],
                    in_offset=bass.IndirectOffsetOnAxis(
                        ap=tokt[:, tch:tch + 1], axis=0))
                tpb = None
                for dd in range(DCH):
                    if dd % 4 == 0:
                        tpb = psum.tile([P, 4, P], bf16, name="etp",
                                        tag="misc", bufs=1)
                    tp = tpb[:, dd % 4, :]
                    nc.tensor.transpose(tp, g[:, dd * P:(dd + 1) * P],
                                        ident_b[:])
                    nc.vector.tensor_add(
                        x[dd][:, tch * P:(tch + 1) * P], tp,
                        peTt[:, dd, tch * P:(tch + 1) * P])
                qkv_stage(0, tch, x, w0, qT, kT, vT)

            # ================= transformer layers =================
            for l in range(L):
                wo_ts = []
                for hp in range(DCH):
                    wt = wop.tile([DK, 2, D], bf16, name=f"wo{l}_{hp}",
                                  tag="wo", bufs=7)
                    nc.scalar.dma_start(wt[:], wo_r[l, hp])
                    wo_ts.append(wt)
                w1_cache = {}
                for og in range(4):
                    wt = w1p.tile([P, DCH, DCH * P], bf16,
                                  name=f"w1_{l}_{og}", tag="w1", bufs=4)
                    nc.scalar.dma_start(wt[:], w1_r[l, og])
                    w1_cache[og] = wt
                if l < L - 1:
                    wn = load_qkv_w(l + 1)
                    qn, kn, vtn = qkvtiles(l + 1)
                    nxt = (qn, kn, vtn, wn)
                else:
                    nxt = xtiles("hT_")
                t1, xn1, t2, xnext = (xtiles(f"t1_{l}"), xtiles(f"xn1_{l}"),
                                      xtiles(f"t2_{l}"), xtiles(f"x{l + 1}_"))

                KT0, V0 = load_kv(l, 0)
                KT1, V1 = load_kv(l, 1)
                with nc.named_scope(f"L{l}attA"):
                    hpA = attn_stage(0, qT, KT0, V0, None, None)
                with nc.named_scope(f"L{l}postA"):
                    post_stage(l, 0, hpA, x, t1, xn1, t2, xnext, wo_ts,
                               w1_cache, nxt)
                with nc.named_scope(f"L{l}attB"):
                    hpB = attn_stage(1, qT, KT0, V0, KT1, V1)
                with nc.named_scope(f"L{l}postB"):
                    post_stage(l, 1, hpB, x, t1, xn1, t2, xnext, wo_ts,
                               w1_cache, nxt)
                x = xnext
                if l < L - 1:
                    qT, kT, vT = nxt[0], nxt[1], nxt[2]

        # ================= vocab-parallel head =================
        with ExitStack() as hctx:
            htp = hctx.enter_context(tc.tile_pool(name="htp", bufs=12))
            wvp = hctx.enter_context(tc.tile_pool(name="wvp", bufs=7))
            otp = hctx.enter_context(tc.tile_pool(name="otp", bufs=3))

            HT = [[None] * DCH for _ in range(2)]
            for half in range(2):
                houtr = houtF[half].rearrange("(j p c) -> j p c", p=P, c=KC)
                for dd in range(DCH):
                    ht = htp.tile([P, NCORE * P], bf16,
                                  name=f"HT{half}_{dd}", tag="ht", bufs=12)
                    nc.sync.dma_start(
                        ht[:].rearrange("p (j t) -> p j t", j=NCORE),
                        houtr[:, :, dd * P:(dd + 1) * P]
                        .rearrange("j p t -> p j t"))
                    HT[half][dd] = ht

            for vh in range(2):
                wts = []
                for dd in range(DCH):
                    wt = wvp.tile([P, VPAD // 2], bf16, name="woutt",
                                  tag="wv", bufs=7)
                    nc.scalar.dma_start(
                        wt[:], woutc[dd * P:(dd + 1) * P,
                                     vh * (VPAD // 2):(vh + 1) * (VPAD // 2)])
                    wts.append(wt)
                for vc in range(VCH // 2):
                    vch = vh * (VCH // 2) + vc
                    ot = otp.tile([P, 2048], f32, name="lsb", tag="lsb",
                                  bufs=3)
                    qi = 0
                    pend = None
                    for half in range(2):
                        for tq in range(2):
                            lp = psum.tile([P, 512], f32, name="logps",
                                           tag="bank", bufs=3)
                            for dd in range(DCH):
                                nc.tensor.matmul(
                                    lp[:],
                                    lhsT=wts[dd][:, vc * P:(vc + 1) * P],
                                    rhs=HT[half][dd][:, tq * 512:
                                                     (tq + 1) * 512],
                                    start=(dd == 0), stop=(dd == DCH - 1),
                                    skip_group_check=True)
                            if pend is not None:
                                plp, posl, pqi = pend
                                if pqi % 2 == 0:
                                    nc.scalar.activation(
                                        posl, plp[:], AF.Identity,
                                        bias=boutt[:, vch:vch + 1])
                                else:
                                    nc.vector.tensor_scalar_add(
                                        posl, plp[:], boutt[:, vch:vch + 1])
                            osl = ot[:, half * 1024 + tq * 512:
                                     half * 1024 + (tq + 1) * 512]
                            pend = (lp, osl, qi + vc)
                            qi += 1
                    plp, posl, pqi = pend
                    if pqi % 2 == 0:
                        nc.scalar.activation(
                            posl, plp[:], AF.Identity,
                            bias=boutt[:, vch:vch + 1])
                    else:
                        nc.vector.tensor_scalar_add(
                            posl, plp[:], boutt[:, vch:vch + 1])
                    eng = nc.sync if vch % 2 == 0 else nc.gpsimd
                    eng.dma_start(out[vch * P:(vch + 1) * P, :], ot[:])

    return nc


_CACHED = {}


def _compiled():
    if "nc" not in _CACHED:
        nc = bacc.Bacc("TRN2", target_bir_lowering=False, debug=False,
                       num_devices=NCORE)
        build(nc)
        nc.compile()
        _CACHED["nc"] = nc
    return _CACHED["nc"]


def _bf(a):
    return np.ascontiguousarray(np.asarray(a, np.float32)).astype(
        ml_dtypes.bfloat16)


def _make_inputs(tokens, emb, pe, wq, bq, wk, bk, wv, bv, wo, bo,
                 w1, b1, w2, b2, g1, be1, g2, be2, gf, bf, wout, bout):
    f = np.float32
    tokens = np.asarray(tokens).astype(np.int32)

    def parr(b):  # [L, dim] -> [L, P, dim//P]
        b = np.asarray(b, f)
        return b.reshape(L, b.shape[1] // P, P).transpose(0, 2, 1)

    def parr1(b):  # [dim] -> [P, dim//P]
        b = np.asarray(b, f)
        return np.ascontiguousarray(b.reshape(b.shape[0] // P, P).T)

    ball = np.concatenate(
        [parr(bq), parr(bk), parr(bv), parr(bo), parr(b2),
         parr(g1), parr(be1), parr(g2), parr(be2), parr(b1)], axis=2)

    # wqkv [L, DCH, P, 3, D]: lhsT chunks, natural rows
    wq_, wk_, wv_ = (np.asarray(w, f).reshape(L, DCH, P, D)
                     for w in (wq, wk, wv))
    wqkv = _bf(np.stack([wq_, wk_, wv_], axis=3))
    wo_r = _bf(np.asarray(wo, f).reshape(L, DCH, 2, DK, D)
               .transpose(0, 1, 3, 2, 4))
    # w1 [L, D, F] -> [L, 4, P(ic-row), DCH(ic), 768]
    w1_ = np.asarray(w1, f).reshape(L, DCH, P, 4, DCH * P)
    w1_r = _bf(w1_.transpose(0, 3, 2, 1, 4))
    w2_r = _bf(np.asarray(w2, f).reshape(L, FCH, P, D))

    emb_s = _bf(np.asarray(emb, f) * SQD)
    pe = np.asarray(pe, f)
    wout = np.asarray(wout, f)
    bout = np.asarray(bout, f)

    common = {
        "embs": emb_s, "wqkv": wqkv, "wo_r": wo_r, "w1_r": w1_r,
        "w2_r": w2_r, "ball": np.ascontiguousarray(ball),
        "gfp": parr1(gf), "bfp": parr1(bf),
    }

    tri = np.where(np.arange(P)[:, None] <= np.arange(P)[None, :],
                   1.0, 0.0).astype(f)  # [k, q]: visible iff k <= q
    zeros = np.ones((P, P), f)
    neg = np.zeros((P, P), f)

    in_maps = []
    for c in range(NCORE):
        b, r = divmod(c, GRP)
        chunks = (r, 7 - r)
        rows = np.concatenate(
            [np.arange(ch * P, (ch + 1) * P) for ch in chunks])
        tok_c = np.stack(
            [tokens[b, ch * P:(ch + 1) * P] for ch in chunks], axis=1
        ).astype(np.int32)
        peT_c = np.ascontiguousarray(pe[rows].T)  # [D, TOK]
        peTr = _bf(peT_c.reshape(DCH, P, TOK).transpose(1, 0, 2))

        # additive masks: amask[0][s] for qc1=chunk r vs key chunk s;
        # amask[1][g] for qc2=chunk 7-r vs key chunk 7-g.
        am = np.empty((2, 4, P, P), f)
        for s in range(4):
            am[0, s] = zeros if s < r else (tri if s == r else neg)
        for g2_ in range(4):
            am[1, g2_] = zeros if g2_ > r else (tri if g2_ == r else neg)

        wslice = np.zeros((D, VPAD), f)
        wslice[:, :VSH] = wout[:, c * VSH:(c + 1) * VSH]
        bslice = np.zeros((VPAD,), f)
        bslice[:VSH] = bout[c * VSH:(c + 1) * VSH]
        boutp_c = np.ascontiguousarray(bslice.reshape(VCH, P).T)

        m = dict(common)
        m.update({
            "tok": tok_c,
            "peTr": peTr,
            "amask": _bf(am),
            "woutc": _bf(wslice),
            "boutp": boutp_c,
        })
        in_maps.append(m)
    return in_maps


def run(in_maps, **kwargs):
    nc = _compiled()
    return run_bass_kernel_spmd(nc, in_maps, list(range(NCORE)), **kwargs)


def assemble(results):
    """results[c]['out'] [VPAD, 8*TOK] -> full logits [B, S, V].

    out col = half*1024 + j*128 + t, where half selects the token chunk
    (rank j owns chunks (j%4, 7-j%4)) and j is the source core.
    """
    full = np.empty((B, S, V), np.float32)
    for c in range(NCORE):
        lt = np.asarray(results[c]["out"])[:VSH]  # [4000, 2048]
        lg = lt.T  # [2048, 4000]
        for j in range(NCORE):
            bj, rj = divmod(j, GRP)
            for half, ch in enumerate((rj, 7 - rj)):
                full[bj, ch * P:(ch + 1) * P, c * VSH:(c + 1) * VSH] = \
                    lg[half * 1024 + j * P:half * 1024 + (j + 1) * P]
    return full


def kernel(**inputs):
    in_maps = _make_inputs(**inputs)
    res = run(in_maps)
    return assemble(res.results)


# revision 29
# speedup vs baseline: 1.2256x; 1.0104x over previous
"""MinimalGPT forward on 8 Trainium2 NeuronCores — v2.

Sharding: sequence-parallel transformer + vocab-parallel head (zigzag).
  core c: batch b=c//4, rank r=c%4, owns seq chunks (r, 7-r) = 2x128 tokens.

v2 changes vs v1:
  - bf16 weights + activations end-to-end (fp32 PSUM accumulate, fp32 LN
    stats); halves HBM + collective bytes, enables 128-wide matmuls at
    full PE rate.
  - causal structure: qc1 (chunk r) attends only to chunks 0-3, qc2 only
    to 0..7-r; additive masks applied as identity-matmuls into the score
    PSUM (uniform SPMD program, per-core mask data).
  - softmax denominators ride the AV matmul (ones column packed into V);
    reciprocal via DVE reciprocal_approx_fast on head-pairs.
  - per-layer kv AllGather split into two (one per token chunk), issued
    as soon as that chunk's kv is projected; attention over chunks 0-3
    starts after AG0, hiding most collective latency behind compute.
  - ACT engine uses a single fn table (exp/ln/square/identity); LN rstd
    computed as exp(-0.5*ln(var+eps)).
"""

import math
import os
import numpy as np
import ml_dtypes
from contextlib import ExitStack

import concourse.bass as bass
import concourse.tile as tile
from concourse import bacc, mybir
from concourse.bass_utils import run_bass_kernel_spmd
from concourse.masks import make_identity

f32 = mybir.dt.float32
bf16 = mybir.dt.bfloat16
i32 = mybir.dt.int32
AF = mybir.ActivationFunctionType
OP = mybir.AluOpType

V, D, H, L, F = 32000, 768, 12, 6, 3072
B, S = 2, 1024
P = 128
DK = 64
DCH = D // P           # 6
FCH = F // P           # 24
TOK = 256              # tokens per core (2 chunks of 128)
NCORE, GRP = 8, 4
VPAD = 4096
VCH = VPAD // P        # 32
VSH = V // NCORE       # 4000
EPS = 1e-5
SQD = math.sqrt(D)
ISQDK = 1.0 / math.sqrt(DK)
HV = H * (DK + 1)      # 780: natural V cols incl per-head ones column
KC = DCH * P           # 768
KVC = KC + HV          # 1548 bounce cols per token chunk

# packed per-layer bias/gain columns in `ball` [L, P, 78]
BQ, BK, BV, BO, B2, G1, BE1, G2, BE2, B1 = 0, 6, 12, 18, 24, 30, 36, 42, 48, 54

KV_GROUPS = [[0, 1, 2, 3], [4, 5, 6, 7]]
ALL_GROUP = [list(range(NCORE))]


def build(nc):
    def din(name, shape, dt=f32):
        return nc.dram_tensor(name, shape, dt, kind="ExternalInput").ap()

    tok = din("tok", [P, 2], i32)
    peTr = din("peTr", [P, DCH, TOK], bf16)
    embs = din("embs", [V, D], bf16)           # pre-scaled by sqrt(D)
    amask = din("amask", [2, 4, P, P], bf16)   # additive score masks
    wqkv = din("wqkv", [L, DCH, P, 3, D], bf16)
    wo_r = din("wo_r", [L, DCH, DK, 2, D], bf16)
    w1_r = din("w1_r", [L, 4, P, DCH, DCH * P], bf16)
    w2_r = din("w2_r", [L, FCH, P, D], bf16)
    ball = din("ball", [L, P, 78])
    gfp = din("gfp", [P, DCH])
    bfp = din("bfp", [P, DCH])
    woutc = din("woutc", [D, VPAD], bf16)
    boutp = din("boutp", [P, VCH])

    out = nc.dram_tensor("out", [VPAD, NCORE * TOK], f32,
                         kind="ExternalOutput").ap()

    kvins = [[nc.dram_tensor(f"kvi{l}_{t}", [P * KVC], bf16).ap()
              for t in range(2)] for l in range(L)]
    kvouts = [[nc.dram_tensor(f"kvout{l}_{t}", [GRP * P * KVC], bf16).ap()
               for t in range(2)] for l in range(L)]
    hinF = [nc.dram_tensor(f"hinF{t}", [P * KC], bf16).ap() for t in range(2)]
    dwi = [nc.dram_tensor(f"dwi{i}", [256], bf16).ap() for i in range(2)]
    dwo = [nc.dram_tensor("dwo0", [256 * GRP], bf16).ap(),
           nc.dram_tensor("dwo1", [256 * NCORE], bf16,
                          addr_space="Shared").ap()]
    houtF = [nc.dram_tensor(f"houtF{t}", [NCORE * P * KC], bf16,
                            addr_space="Shared").ap() for t in range(2)]

    with tile.TileContext(
            nc, trace_sim=os.environ.get("TRACE_SIM", "0") == "1",
    ) as tc, ExitStack() as octx, \
            nc.allow_low_precision(reason="bf16 datapath, fp32 accumulate"):
        const = octx.enter_context(tc.tile_pool(name="const", bufs=1))
        stats = octx.enter_context(tc.tile_pool(name="stats", bufs=10))
        # PSUM: 8 bank-slots total (every slot pads to a full 2KB bank):
        # bank(2) scores/logits, oT(1), ga(2) qkv/wo outs, yps(2), misc(1)
        psum = octx.enter_context(
            tc.tile_pool(name="psum", bufs=1, space="PSUM"))

        def ctile(shape, dt, nm):
            return const.tile(shape, dt, name=nm, tag=nm)

        ident_f = ctile([P, P], f32, "ident_f")
        make_identity(nc, ident_f[:])
        ident_b = ctile([P, P], bf16, "ident_b")
        nc.vector.tensor_copy(ident_b[:], ident_f[:])
        ones_col_b = ctile([P, 1], bf16, "ones_col_b")
        nc.vector.memset(ones_col_b[:], 1.0)
        ones_row_b = ctile([1, P], bf16, "ones_row_b")
        nc.vector.memset(ones_row_b[:], 1.0)
        zrow = ctile([1, 4 * P], bf16, "zrow")
        nc.vector.memset(zrow[:], 0.0)
        one_i = ctile([1, 1], i32, "one_i")
        nc.vector.memset(one_i[:], 1)
        magic_row = ctile([1, P], i32, "magic_row")
        nc.vector.memset(magic_row[:], 0x5F3759DF)
        eps_t = ctile([1, 1], f32, "eps_t")
        nc.vector.memset(eps_t[:], EPS)
        # tiny dummy collectives to warm the CC stack while embedding runs
        nc.gpsimd.collective_compute(
            "AllGather", OP.bypass, replica_groups=KV_GROUPS,
            ins=[dwi[0].opt()], outs=[dwo[0].opt()])
        nc.gpsimd.collective_compute(
            "AllGather", OP.bypass, replica_groups=ALL_GROUP,
            ins=[dwi[1].opt()], outs=[dwo[1].opt()])
        tokt = ctile([P, 2], i32, "tokt")
        nc.sync.dma_start(tokt[:], tok[:])
        mt = []
        for qi in range(2):
            row = []
            for s in range(4):
                m = ctile([P, P], bf16, f"mask{qi}_{s}")
                nc.sync.dma_start(m[:], amask[qi, s])
                row.append(m)
            mt.append(row)
        peTt = ctile([P, DCH, TOK], bf16, "peTt")
        nc.sync.dma_start(peTt[:], peTr[:])
        gft = ctile([P, DCH], f32, "gft")
        nc.sync.dma_start(gft[:], gfp[:])
        bft = ctile([P, DCH], f32, "bft")
        nc.sync.dma_start(bft[:], bfp[:])
        boutt = ctile([P, VCH], f32, "boutt")
        nc.sync.dma_start(boutt[:], boutp[:])

        with ExitStack() as lctx:
            acts = lctx.enter_context(tc.tile_pool(name="acts", bufs=34))
            sqp = lctx.enter_context(tc.tile_pool(name="sqp", bufs=4))
            bcp = lctx.enter_context(tc.tile_pool(name="bcp", bufs=4))
            qkvp = lctx.enter_context(tc.tile_pool(name="qkvp", bufs=6))
            vna = lctx.enter_context(tc.tile_pool(name="vna", bufs=3))
            ktp = lctx.enter_context(tc.tile_pool(name="ktp", bufs=9))
            vp = lctx.enter_context(tc.tile_pool(name="vp", bufs=9))
            ep = lctx.enter_context(tc.tile_pool(name="ep", bufs=3))
            hpp = lctx.enter_context(tc.tile_pool(name="hpp", bufs=8))
            ftp = lctx.enter_context(tc.tile_pool(name="ftp", bufs=6))
            wqp = lctx.enter_context(tc.tile_pool(name="wqp", bufs=7))
            wop = lctx.enter_context(tc.tile_pool(name="wop", bufs=7))
            w1p = lctx.enter_context(tc.tile_pool(name="w1p", bufs=4))
            w2p = lctx.enter_context(tc.tile_pool(name="w2p", bufs=6))
            bpool = lctx.enter_context(tc.tile_pool(name="bpool", bufs=3))

            ballts = {}

            def get_ball(l):
                if l not in ballts:
                    t = bpool.tile([P, 78], f32, name=f"ball{l}", tag="ball",
                                   bufs=3)
                    nc.sync.dma_start(t[:], ball[l])
                    ballts[l] = t
                return ballts[l]

            def load_qkv_w(l):
                wts = []
                for ic in range(DCH):
                    wt = wqp.tile([P, 3, D], bf16, name=f"wqkv{l}_{ic}",
                                  tag="wqkv", bufs=7)
                    nc.scalar.dma_start(wt[:], wqkv[l, ic])
                    wts.append(wt)
                return wts

            def _proj(pi, bcol, dst, xin, wts, ballt, tc0, tc1):
                gaA = psum.tile([P, 4, P], f32, name="gaA", tag="bank",
                                bufs=3)
                gaB = psum.tile([P, 4, P], f32, name="gaB", tag="bank",
                                bufs=3)
                outs = [gaA[:, oc, :] if oc < 4 else gaB[:, oc - 4, :]
                        for oc in range(DCH)]
                for oc in range(DCH):
                    for ic in range(DCH):
                        nc.tensor.matmul(
                            outs[oc],
                            lhsT=wts[ic][:, pi, oc * P:(oc + 1) * P],
                            rhs=xin[ic][:, tc0:tc1],
                            start=(ic == 0), stop=(ic == DCH - 1),
                            skip_group_check=True,
                        )
                for oc in range(DCH):
                    nc.vector.tensor_scalar_add(
                        dst[:, oc, tc0:tc1], outs[oc],
                        ballt[:, bcol + oc:bcol + oc + 1])

            def qkv_stage(l, tch, xin, wts, qTt, kTt, vTt):
                """Project x(tch) -> k,v first, bounce+AllGather, then q."""
                ballt = get_ball(l)
                tc0, tc1 = tch * P, (tch + 1) * P
                _proj(1, BK, kTt, xin, wts, ballt, tc0, tc1)
                _proj(2, BV, vTt, xin, wts, ballt, tc0, tc1)

                # natural V (+ ones cols) for this token chunk
                vn = vna.tile([P, HV], bf16, name=f"vn{l}_{tch}", tag="vn",
                              bufs=3)
                nc.gpsimd.memset(
                    vn[:].rearrange("p (h c) -> p h c", h=H)[:, :, DK:], 1.0)
                tpb = None
                for dd in range(DCH):
                    if dd % 4 == 0:
                        tpb = psum.tile([P, 4, P], bf16, name="vtp",
                                        tag="misc", bufs=1)
                    tp = tpb[:, dd % 4, :]
                    nc.tensor.transpose(tp, vTt[:, dd, tc0:tc1], ident_b[:])
                    for j in range(2):
                        h = 2 * dd + j
                        nc.scalar.activation(
                            vn[:, h * (DK + 1):h * (DK + 1) + DK],
                            tp[:, j * DK:(j + 1) * DK], AF.Identity)
                bounce_kv(l, tch, kTt, vn)
                _proj(0, BQ, qTt, xin, wts, ballt, tc0, tc1)

            def bounce_kv(l, tch, kTt, vn):
                kvi = kvins[l][tch].rearrange("(p c) -> p c", c=KVC)
                nc.sync.dma_start(
                    kvi[:, 0:KC].rearrange("p (d t) -> p d t", d=DCH),
                    kTt[:, :, tch * P:(tch + 1) * P])
                nc.sync.dma_start(kvi[:, KC:], vn[:])
                nc.gpsimd.collective_compute(
                    "AllGather", OP.bypass, replica_groups=KV_GROUPS,
                    ins=[kvins[l][tch].opt()], outs=[kvouts[l][tch].opt()])

            def load_kv(l, half):
                kvo = kvouts[l][half].rearrange("(g p c) -> g p c", p=P, c=KVC)
                KTs, Vs = [], []
                for g in range(GRP):
                    kt = ktp.tile([P, DCH, P], bf16, name=f"KT{half}_{g}",
                                  tag="kt", bufs=9)
                    nc.sync.dma_start(
                        kt[:], kvo[g, :, 0:KC].rearrange(
                            "p (d t) -> p d t", d=DCH))
                    KTs.append(kt)
                    v = vp.tile([P, HV], bf16, name=f"V{half}_{g}", tag="v",
                                bufs=9)
                    nc.sync.dma_start(v[:], kvo[g, :, KC:])
                    Vs.append(v)
                return KTs, Vs

            def attn_stage(qc, qTt, KT0, V0, KT1, V1):
                """Attention for query chunk qc, software-pipelined: the
                score matmuls of unit i+1 are emitted before the AV matmuls
                of unit i so the PE never waits on the exp (ACT) feedback."""
                qc0, qc1_ = qc * P, (qc + 1) * P
                hpTs = []
                aoTs = {}

                def build(hp, sub):
                    h = 2 * hp + sub
                    hr = sub * DK
                    banks = []
                    bank0 = psum.tile([P, 4 * P], f32, name="sc0",
                                      tag="bank", bufs=3)
                    for s in range(GRP):
                        nc.tensor.matmul(
                            bank0[:, s * P:(s + 1) * P],
                            lhsT=KT0[s][hr:hr + DK, hp, :],
                            rhs=qTt[hr:hr + DK, hp, qc0:qc1_],
                            start=True, stop=True,
                            skip_group_check=True)
                    banks.append((bank0, V0, mt[0] if qc == 0 else None))
                    if qc == 1:
                        bank1 = psum.tile([P, 4 * P], f32, name="sc1",
                                          tag="bank", bufs=3)
                        for s in range(GRP):
                            nc.tensor.matmul(
                                bank1[:, s * P:(s + 1) * P],
                                lhsT=KT1[s][hr:hr + DK, hp, :],
                                rhs=qTt[hr:hr + DK, hp, qc0:qc1_],
                                start=True, stop=True,
                                skip_group_check=True)
                        banks.append((bank1, V1, mt[1]))
                    es = []
                    for bank, _vs, msk in banks:
                        e = ep.tile([P, 4 * P], bf16, name="e", tag="e",
                                    bufs=3)
                        nc.scalar.activation(e[:], bank[:], AF.Exp,
                                             scale=ISQDK)
                        if msk is not None:
                            for s in range(GRP):
                                eng = nc.vector if s % 2 == 0 else nc.gpsimd
                                eng.tensor_mul(
                                    e[:, s * P:(s + 1) * P],
                                    e[:, s * P:(s + 1) * P], msk[s][:])
                        es.append(e)
                    return (hp, sub, banks, es)

                def do_av(item):
                    hp, sub, banks, es = item
                    h = 2 * hp + sub
                    if sub == 0:
                        aoTs[hp] = psum.tile([P, 4, P], f32, name="aoT",
                                             tag="aoT", bufs=2)
                    oT = aoTs[hp][0:DK + 1, sub, :]
                    nbl = len(banks) * GRP
                    bi = 0
                    for (bank, vs, _m), e in zip(banks, es):
                        for s in range(GRP):
                            nc.tensor.matmul(
                                oT,
                                lhsT=vs[s][:, h * (DK + 1):
                                           (h + 1) * (DK + 1)],
                                rhs=e[:, s * P:(s + 1) * P],
                                start=(bi == 0), stop=(bi == nbl - 1),
                                skip_group_check=True)
                            bi += 1

                def finish(hp):
                    aoT = aoTs.pop(hp)
                    oT2 = aoT[0:DK + 1, 0:2, :]
                    den = stats.tile([1, 2, P], f32, name="den", tag="st")
                    nc.vector.tensor_copy(den[:, 0, :],
                                          oT2[DK:DK + 1, 0, :])
                    nc.vector.tensor_copy(den[:, 1, :],
                                          oT2[DK:DK + 1, 1, :])
                    rec = stats.tile([1, 2 * P], f32, name="rec", tag="st")
                    nc.vector.reciprocal_approx_fast(
                        rec[:], den[:].rearrange("a s p -> a (s p)"))
                    recb = stats.tile([1, 2 * P], bf16, name="recb",
                                      tag="st")
                    nc.vector.tensor_copy(recb[:], rec[:])
                    rb = aoT[0:DK, 2:4, :]
                    nc.tensor.matmul(rb, lhsT=ones_row_b[:, 0:DK],
                                     rhs=recb[:], start=True, stop=True,
                                     skip_group_check=True)
                    rbs = bcp.tile([DK, 2 * P], bf16, name="rbs", tag="bc",
                                   bufs=4)
                    nc.vector.tensor_copy(
                        rbs[:], rb.rearrange("p a b -> p (a b)"))
                    for sub in range(2):
                        oh = hpp.tile([DK, P], bf16, name=f"oh{hp}_{sub}",
                                      tag="oh", bufs=14)
                        nc.vector.tensor_mul(oh[:], oT2[0:DK, sub, :],
                                             rbs[:, sub * P:(sub + 1) * P])
                        hpTs.append(oh)
                    hpTs.append(None)

                prev = None
                for hp in range(DCH):
                    for sub in range(2):
                        item = build(hp, sub)
                        if prev is not None:
                            do_av(prev)
                            if prev[1] == 1:
                                finish(prev[0])
                        prev = item
                do_av(prev)
                finish(prev[0])
                return hpTs

            def ln_stage(tch, tin, g_ap, gcol, be_ap, becol, tout):
                """LayerNorm over features for one token chunk.

                tin: 6 [P, TOK] bf16 tiles (reads [:, tch*P:+P]);
                tout: 6 tiles (writes same slice).
                """
                tc0, tc1 = tch * P, (tch + 1) * P
                stp = psum.tile([1, 2, P], f32, name="stp", tag="misc",
                                bufs=1)
                st_s = stp[:, 0, :]
                st_q = stp[:, 1, :]
                sqs = []
                for dd in range(DCH):
                    sq = sqp.tile([P, P], bf16, name="sq", tag="sq", bufs=6)
                    nc.scalar.activation(sq[:], tin[dd][:, tc0:tc1], AF.Square)
                    sqs.append(sq)
                for dd in range(DCH):
                    nc.tensor.matmul(st_s, lhsT=ones_col_b[:],
                                     rhs=tin[dd][:, tc0:tc1],
                                     start=(dd == 0), stop=(dd == DCH - 1),
                                     skip_group_check=True)
                for dd in range(DCH):
                    nc.tensor.matmul(st_q, lhsT=ones_col_b[:], rhs=sqs[dd][:],
                                     start=(dd == 0), stop=(dd == DCH - 1),
                                     skip_group_check=True)
                nm = stats.tile([1, P], bf16, name="nm", tag="st")
                nc.vector.tensor_scalar_mul(nm[:], st_s, -1.0 / D)
                m2 = stats.tile([1, P], f32, name="m2", tag="st")
                nc.vector.tensor_mul(m2[:], nm[:], nm[:])
                ex2 = stats.tile([1, P], f32, name="ex2", tag="st")
                nc.vector.tensor_scalar_mul(ex2[:], st_q, 1.0 / D)
                ve = stats.tile([1, P], f32, name="ve", tag="st")
                nc.vector.tensor_sub(ve[:], ex2[:], m2[:])
                nc.vector.tensor_scalar_add(ve[:], ve[:], EPS)
                vh = stats.tile([1, P], f32, name="vh", tag="st")
                nc.vector.tensor_scalar_mul(vh[:], ve[:], 0.5)
                yi = stats.tile([1, P], i32, name="yi", tag="st")
                nc.vector.tensor_scalar(yi[:], ve[:].bitcast(i32), one_i[:],
                                        None, op0=OP.arith_shift_right)
                nc.vector.tensor_sub(yi[:], magic_row[:], yi[:])
                y = yi[:].bitcast(f32)
                t = stats.tile([1, P], f32, name="t", tag="st")
                a = stats.tile([1, P], f32, name="a", tag="st")
                for _ in range(2):
                    nc.vector.tensor_mul(t[:], y, y)
                    nc.vector.tensor_mul(t[:], t[:], vh[:])
                    nc.vector.tensor_scalar(a[:], t[:], -1.0, 1.5,
                                            op0=OP.mult, op1=OP.add)
                    nc.vector.tensor_mul(y, a[:], y)
                rstd = stats.tile([1, P], bf16, name="rstd", tag="st")
                nc.vector.tensor_copy(rstd[:], y)
                bcps = psum.tile([P, 2, P], f32, name="bcps", tag="misc",
                                 bufs=1)
                nc.tensor.matmul(bcps[:, 0, :], lhsT=ones_row_b[:], rhs=nm[:],
                                 start=True, stop=True, skip_group_check=True)
                nc.tensor.matmul(bcps[:, 1, :], lhsT=ones_row_b[:],
                                 rhs=rstd[:],
                                 start=True, stop=True, skip_group_check=True)
                nmb = bcp.tile([P, P], bf16, name="nmbb", tag="bc", bufs=4)
                nc.vector.tensor_copy(nmb[:], bcps[:, 0, :])
                rsb = bcp.tile([P, P], bf16, name="rsbb", tag="bc", bufs=4)
                nc.vector.tensor_copy(rsb[:], bcps[:, 1, :])
                for dd in range(DCH):
                    eng = nc.vector if dd % 2 == 0 else nc.gpsimd
                    osl = tout[dd][:, tc0:tc1]
                    eng.tensor_add(osl, tin[dd][:, tc0:tc1], nmb[:])
                    eng.tensor_mul(osl, osl, rsb[:])
                    eng.tensor_scalar(
                        osl, osl, g_ap[:, gcol + dd:gcol + dd + 1],
                        be_ap[:, becol + dd:becol + dd + 1],
                        op0=OP.mult, op1=OP.add)

            def post_stage(l, tch, hpTs, x, t1, xn1, t2, xnext, wo_ts,
                           w1_cache, nxt):
                """wo+res+ln1+FFN+res+ln2 for one token chunk, then either
                qkv for layer l+1 (+ kv bounce/AG) or final LN + hT bounce."""
                ballt = get_ball(l)
                tc0, tc1 = tch * P, (tch + 1) * P
                # ---- Wo + residual ----
                gaA = psum.tile([P, 4, P], f32, name="woA", tag="bank",
                                bufs=3)
                gaB = psum.tile([P, 4, P], f32, name="woB", tag="bank",
                                bufs=3)
                ops_ = [gaA[:, oc, :] if oc < 4 else gaB[:, oc - 4, :]
                        for oc in range(DCH)]
                for oc in range(DCH):
                    for hp in range(DCH):
                        for sub in range(2):
                            oh = hpTs[hp * 3 + sub]
                            nc.tensor.matmul(
                                ops_[oc],
                                lhsT=wo_ts[hp][:, sub, oc * P:(oc + 1) * P],
                                rhs=oh[:],
                                start=(hp == 0 and sub == 0),
                                stop=(hp == DCH - 1 and sub == 1),
                                skip_group_check=True)
                for oc in range(DCH):
                    nc.vector.scalar_tensor_tensor(
                        t1[oc][:, tc0:tc1], ops_[oc],
                        ballt[:, BO + oc:BO + oc + 1], x[oc][:, tc0:tc1],
                        op0=OP.add, op1=OP.add)
                ln_stage(tch, t1, ballt, G1, ballt, BE1, xn1)
                # ---- FFN ----
                ypA = psum.tile([P, 4, P], f32, name="ypA", tag="yps",
                                bufs=2)
                ypB = psum.tile([P, 4, P], f32, name="ypB", tag="yps",
                                bufs=2)
                yps = [ypA[:, oc, :] if oc < 4 else ypB[:, oc - 4, :]
                       for oc in range(DCH)]
                for yp in (ypA, ypB):
                    nc.tensor.matmul(
                        yp[:].rearrange("p a b -> p (a b)"),
                        lhsT=ones_row_b[:], rhs=zrow[:],
                        start=True, stop=True, skip_group_check=True)
                fpb = None
                prevf = None
                for og in range(4):
                    w1t = w1_cache[og]
                    for j in range(DCH):
                        fc = og * DCH + j
                        if fc % 2 == 0:
                            w2t = w2p.tile([P, 2, D], bf16, name="w2t",
                                           tag="w2", bufs=6)
                            nc.gpsimd.dma_start(w2t[:], w2_r[l, fc:fc + 2]
                                                .rearrange("f p d -> p f d"))
                        if fc % 4 == 0:
                            fpb = psum.tile([P, 4, P], f32, name="fpb",
                                            tag="bank", bufs=3)
                        fps = fpb[:, fc % 4, :]
                        for ic in range(DCH):
                            nc.tensor.matmul(
                                fps,
                                lhsT=w1t[:, ic, j * P:(j + 1) * P],
                                rhs=xn1[ic][:, tc0:tc1],
                                start=(ic == 0), stop=(ic == DCH - 1),
                                skip_group_check=True)
                        ft = ftp.tile([P, P], bf16, name="ft", tag="ft",
                                      bufs=6)
                        if fc % 2 == 0:
                            nc.vector.tensor_scalar(
                                ft[:], fps, ballt[:, B1 + fc:B1 + fc + 1],
                                0.0, op0=OP.add, op1=OP.max)
                        else:
                            nc.scalar.activation(
                                ft[:], fps, AF.Relu,
                                bias=ballt[:, B1 + fc:B1 + fc + 1])
                        if prevf is not None:
                            pfc, pft, pw2t = prevf
                            for oc in range(DCH):
                                nc.tensor.matmul(
                                    yps[oc],
                                    lhsT=pw2t[:, pfc % 2,
                                              oc * P:(oc + 1) * P],
                                    rhs=pft[:],
                                    start=False, stop=False,
                                    skip_group_check=True)
                        prevf = (fc, ft, w2t)
                pfc, pft, pw2t = prevf
                for oc in range(DCH):
                    nc.tensor.matmul(
                        yps[oc],
                        lhsT=pw2t[:, pfc % 2, oc * P:(oc + 1) * P],
                        rhs=pft[:],
                        start=False, stop=True,
                        skip_group_check=True)
                for oc in range(DCH):
                    nc.vector.scalar_tensor_tensor(
                        t2[oc][:, tc0:tc1], yps[oc],
                        ballt[:, B2 + oc:B2 + oc + 1], xn1[oc][:, tc0:tc1],
                        op0=OP.add, op1=OP.add)
                ln_stage(tch, t2, ballt, G2, ballt, BE2, xnext)
                # ---- next-layer qkv or final LN ----
                if l < L - 1:
                    qn, kn, vtn, wn = nxt
                    qkv_stage(l + 1, tch, xnext, wn, qn, kn, vtn)
                else:
                    hT = nxt
                    ln_stage(tch, xnext, gft, 0, bft, 0, hT)
                    hinr = hinF[tch].rearrange("(p c) -> p c", c=KC) \
                        .rearrange("p (d t) -> p d t", d=DCH)
                    for dd in range(DCH):
                        nc.sync.dma_start(hinr[:, dd, :],
                                          hT[dd][:, tc0:tc1])
                    nc.gpsimd.collective_compute(
                        "AllGather", OP.bypass, replica_groups=ALL_GROUP,
                        ins=[hinF[tch].opt()], outs=[houtF[tch].opt()])

            def xtiles(nm):
                return [acts.tile([P, TOK], bf16, name=f"{nm}{d}", tag="x",
                                  bufs=34) for d in range(DCH)]

            def qkvtiles(l):
                return [qkvp.tile([P, DCH, TOK], bf16, name=f"{nm}{l}",
                                  tag="qkv", bufs=6)
                        for nm in ("qT", "kT", "vT")]

            # ================= embedding + layer-0 qkv =================
            x = xtiles("x0_")
            w0 = load_qkv_w(0)
            qT, kT, vT = qkvtiles(0)
            for tch in range(2):
                g = sqp.tile([P, D], bf16, name="embrow", tag="emb", bufs=2)
                nc.gpsimd.indirect_dma_start(
                    out=g[:], out_offset=None, in_=embs[:],
                    in_offset=bass.IndirectOffsetOnAxis(
                        ap=tokt[:, tch:tch + 1], axis=0))
                tpb = None
                for dd in range(DCH):
                    if dd % 4 == 0:
                        tpb = psum.tile([P, 4, P], bf16, name="etp",
                                        tag="misc", bufs=1)
                    tp = tpb[:, dd % 4, :]
                    nc.tensor.transpose(tp, g[:, dd * P:(dd + 1) * P],
                                        ident_b[:])
                    nc.vector.tensor_add(
                        x[dd][:, tch * P:(tch + 1) * P], tp,
                        peTt[:, dd, tch * P:(tch + 1) * P])
                qkv_stage(0, tch, x, w0, qT, kT, vT)

            # ================= transformer layers =================
            for l in range(L):
                wo_ts = []
                for hp in range(DCH):
                    wt = wop.tile([DK, 2, D], bf16, name=f"wo{l}_{hp}",
                                  tag="wo", bufs=7)
                    nc.scalar.dma_start(wt[:], wo_r[l, hp])
                    wo_ts.append(wt)
                w1_cache = {}
                for og in range(4):
                    wt = w1p.tile([P, DCH, DCH * P], bf16,
                                  name=f"w1_{l}_{og}", tag="w1", bufs=4)
                    nc.scalar.dma_start(wt[:], w1_r[l, og])
                    w1_cache[og] = wt
                if l < L - 1:
                    wn = load_qkv_w(l + 1)
                    qn, kn, vtn = qkvtiles(l + 1)
                    nxt = (qn, kn, vtn, wn)
                else:
                    nxt = xtiles("hT_")
                t1, xn1, t2, xnext = (xtiles(f"t1_{l}"), xtiles(f"xn1_{l}"),
                                      xtiles(f"t2_{l}"), xtiles(f"x{l + 1}_"))

                KT0, V0 = load_kv(l, 0)
                KT1, V1 = load_kv(l, 1)
                with nc.named_scope(f"L{l}attA"):
                    hpA = attn_stage(0, qT, KT0, V0, None, None)
                with nc.named_scope(f"L{l}postA"):
                    post_stage(l, 0, hpA, x, t1, xn1, t2, xnext, wo_ts,
                               w1_cache, nxt)
                with nc.named_scope(f"L{l}attB"):
                    hpB = attn_stage(1, qT, KT0, V0, KT1, V1)
                with nc.named_scope(f"L{l}postB"):
                    post_stage(l, 1, hpB, x, t1, xn1, t2, xnext, wo_ts,
                               w1_cache, nxt)
                x = xnext
                if l < L - 1:
                    qT, kT, vT = nxt[0], nxt[1], nxt[2]

        # ================= vocab-parallel head =================
        with ExitStack() as hctx:
            htp = hctx.enter_context(tc.tile_pool(name="htp", bufs=12))
            wvp = hctx.enter_context(tc.tile_pool(name="wvp", bufs=7))
            otp = hctx.enter_context(tc.tile_pool(name="otp", bufs=3))

            HT = [[None] * DCH for _ in range(2)]
            for half in range(2):
                houtr = houtF[half].rearrange("(j p c) -> j p c", p=P, c=KC)
                for dd in range(DCH):
                    ht = htp.tile([P, NCORE * P], bf16,
                                  name=f"HT{half}_{dd}", tag="ht", bufs=12)
                    nc.sync.dma_start(
                        ht[:].rearrange("p (j t) -> p j t", j=NCORE),
                        houtr[:, :, dd * P:(dd + 1) * P]
                        .rearrange("j p t -> p j t"))
                    HT[half][dd] = ht

            for vh in range(2):
                wts = []
                for dd in range(DCH):
                    wt = wvp.tile([P, VPAD // 2], bf16, name="woutt",
                                  tag="wv", bufs=7)
                    nc.scalar.dma_start(
                        wt[:], woutc[dd * P:(dd + 1) * P,
                                     vh * (VPAD // 2):(vh + 1) * (VPAD // 2)])
                    wts.append(wt)
                for vc in range(VCH // 2):
                    vch = vh * (VCH // 2) + vc
                    ot = otp.tile([P, 2048], f32, name="lsb", tag="lsb",
                                  bufs=3)
                    qi = 0
                    pend = None
                    for half in range(2):
                        for tq in range(2):
                            lp = psum.tile([P, 512], f32, name="logps",
                                           tag="bank", bufs=3)
                            for dd in range(DCH):
                                nc.tensor.matmul(
                                    lp[:],
                                    lhsT=wts[dd][:, vc * P:(vc + 1) * P],
                                    rhs=HT[half][dd][:, tq * 512:
                                                     (tq + 1) * 512],
                                    start=(dd == 0), stop=(dd == DCH - 1),
                                    skip_group_check=True)
                            if pend is not None:
                                plp, posl, pqi = pend
                                if pqi % 2 == 0:
                                    nc.scalar.activation(
                                        posl, plp[:], AF.Identity,
                                        bias=boutt[:, vch:vch + 1])
                                else:
                                    nc.vector.tensor_scalar_add(
                                        posl, plp[:], boutt[:, vch:vch + 1])
                            osl = ot[:, half * 1024 + tq * 512:
                                     half * 1024 + (tq + 1) * 512]
                            pend = (lp, osl, qi + vc)
                            qi += 1
                    plp, posl, pqi = pend
                    if pqi % 2 == 0:
                        nc.scalar.activation(
                            posl, plp[:], AF.Identity,
                            bias=boutt[:, vch:vch + 1])
                    else:
                        nc.vector.tensor_scalar_add(
                            posl, plp[:], boutt[:, vch:vch + 1])
                    eng = nc.sync if vch % 2 == 0 else nc.gpsimd
                    eng.dma_start(out[vch * P:(vch + 1) * P, :], ot[:])

    return nc


_CACHED = {}


def _compiled():
    if "nc" not in _CACHED:
        nc = bacc.Bacc("TRN2", target_bir_lowering=False, debug=False,
                       num_devices=NCORE)
        build(nc)
        nc.compile()
        _CACHED["nc"] = nc
    return _CACHED["nc"]


def _bf(a):
    return np.ascontiguousarray(np.asarray(a, np.float32)).astype(
        ml_dtypes.bfloat16)


def _make_inputs(tokens, emb, pe, wq, bq, wk, bk, wv, bv, wo, bo,
                 w1, b1, w2, b2, g1, be1, g2, be2, gf, bf, wout, bout):
    f = np.float32
    tokens = np.asarray(tokens).astype(np.int32)

    def parr(b):  # [L, dim] -> [L, P, dim//P]
        b = np.asarray(b, f)
        return b.reshape(L, b.shape[1] // P, P).transpose(0, 2, 1)

    def parr1(b):  # [dim] -> [P, dim//P]
        b = np.asarray(b, f)
        return np.ascontiguousarray(b.reshape(b.shape[0] // P, P).T)

    ball = np.concatenate(
        [parr(bq), parr(bk), parr(bv), parr(bo), parr(b2),
         parr(g1), parr(be1), parr(g2), parr(be2), parr(b1)], axis=2)

    # wqkv [L, DCH, P, 3, D]: lhsT chunks, natural rows
    wq_, wk_, wv_ = (np.asarray(w, f).reshape(L, DCH, P, D)
                     for w in (wq, wk, wv))
    wqkv = _bf(np.stack([wq_, wk_, wv_], axis=3))
    wo_r = _bf(np.asarray(wo, f).reshape(L, DCH, 2, DK, D)
               .transpose(0, 1, 3, 2, 4))
    # w1 [L, D, F] -> [L, 4, P(ic-row), DCH(ic), 768]
    w1_ = np.asarray(w1, f).reshape(L, DCH, P, 4, DCH * P)
    w1_r = _bf(w1_.transpose(0, 3, 2, 1, 4))
    w2_r = _bf(np.asarray(w2, f).reshape(L, FCH, P, D))

    emb_s = _bf(np.asarray(emb, f) * SQD)
    pe = np.asarray(pe, f)
    wout = np.asarray(wout, f)
    bout = np.asarray(bout, f)

    common = {
        "embs": emb_s, "wqkv": wqkv, "wo_r": wo_r, "w1_r": w1_r,
        "w2_r": w2_r, "ball": np.ascontiguousarray(ball),
        "gfp": parr1(gf), "bfp": parr1(bf),
    }

    tri = np.where(np.arange(P)[:, None] <= np.arange(P)[None, :],
                   1.0, 0.0).astype(f)  # [k, q]: visible iff k <= q
    zeros = np.ones((P, P), f)
    neg = np.zeros((P, P), f)

    in_maps = []
    for c in range(NCORE):
        b, r = divmod(c, GRP)
        chunks = (r, 7 - r)
        rows = np.concatenate(
            [np.arange(ch * P, (ch + 1) * P) for ch in chunks])
        tok_c = np.stack(
            [tokens[b, ch * P:(ch + 1) * P] for ch in chunks], axis=1
        ).astype(np.int32)
        peT_c = np.ascontiguousarray(pe[rows].T)  # [D, TOK]
        peTr = _bf(peT_c.reshape(DCH, P, TOK).transpose(1, 0, 2))

        # additive masks: amask[0][s] for qc1=chunk r vs key chunk s;
        # amask[1][g] for qc2=chunk 7-r vs key chunk 7-g.
        am = np.empty((2, 4, P, P), f)
        for s in range(4):
            am[0, s] = zeros if s < r else (tri if s == r else neg)
        for g2_ in range(4):
            am[1, g2_] = zeros if g2_ > r else (tri if g2_ == r else neg)

        wslice = np.zeros((D, VPAD), f)
        wslice[:, :VSH] = wout[:, c * VSH:(c + 1) * VSH]
        bslice = np.zeros((VPAD,), f)
        bslice[:VSH] = bout[c * VSH:(c + 1) * VSH]
        boutp_c = np.ascontiguousarray(bslice.reshape(VCH, P).T)

        m = dict(common)
        m.update({
            "tok": tok_c,
            "peTr": peTr,
            "amask": _bf(am),
            "woutc": _bf(wslice),
            "boutp": boutp_c,
        })
        in_maps.append(m)
    return in_maps


def run(in_maps, **kwargs):
    nc = _compiled()
    return run_bass_kernel_spmd(nc, in_maps, list(range(NCORE)), **kwargs)


def assemble(results):
    """results[c]['out'] [VPAD, 8*TOK] -> full logits [B, S, V].

    out col = half*1024 + j*128 + t, where half selects the token chunk
    (rank j owns chunks (j%4, 7-j%4)) and j is the source core.
    """
    full = np.empty((B, S, V), np.float32)
    for c in range(NCORE):
        lt = np.asarray(results[c]["out"])[:VSH]  # [4000, 2048]
        lg = lt.T  # [2048, 4000]
        for j in range(NCORE):
            bj, rj = divmod(j, GRP)
            for half, ch in enumerate((rj, 7 - rj)):
                full[bj, ch * P:(ch + 1) * P, c * VSH:(c + 1) * VSH] = \
                    lg[half * 1024 + j * P:half * 1024 + (j + 1) * P]
    return full


def kernel(**inputs):
    in_maps = _make_inputs(**inputs)
    res = run(in_maps)
    return assemble(res.results)
